# revision 40
# baseline (speedup 1.0000x reference)
"""Trainium2 Bass kernel for nn_AUAttnProcessor (self-attn + AU cross-attn + gated fusion).

Sharding: 8 cores = 4 batches x 2 sequence-halves. Each core computes its
1024 query rows end-to-end in a feature-major ("transposed", [D, tokens])
dataflow. k/v are computed locally per half and AllGathered within the
2-core batch pair.

Numerics (validated vs reference on CPU, rel ~2.6e-3):
- bf16: x, w_q/w_k, q/k, logits, whole AU attention (w_av host-scaled x8
  so au_hs matches hs scale), hs, fused, w_out, out projection.
- fp8e4m3: v path (x8 input copy, w_v host-scaled x16, v, v_aug, its
  AllGather payload), main-attention probs.
- Main softmax: exp(SCALE*logits - 2) -- the -2 keeps probs under fp8 max
  (z reaches ~6.7); the factor e^-2 cancels exactly through the ones-column
  normalizer. AU probs stay bf16 (temperature-5 logits reach e^26).
- PV runs fp8 DoubleRow: probs tiles hold kc pairs [128, 2, R] so one
  matmul contracts 256 keys at 0.5 cycles/col.
- gate: sigmoid(x) = 0.5*(1+tanh(x/2)) -- Tanh shares the Exp activation
  table, so the only table switches are around the 3 Silu instructions.
  fused = (1+tanh)*au_hs8 + hs16 via one scalar_tensor_tensor per head.
- out = fused(16x) @ w_out -> evac (psum * 1/16 + residual + b_out).

Schedule: window is ACT(exp)-paced (~157us of activation work). k proj ->
AGk -> q h0 -> attention starts on kT slot 0 while everything else (q h1-7,
v proj, AU, gate, out-proj partials) fills PE slack under the exp window.
"""

import numpy as np

import concourse.bacc as bacc
import concourse.bass as bass
import concourse.tile as tile
from concourse import mybir
from concourse.bass_utils import run_bass_kernel_spmd

F32 = mybir.dt.float32
BF16 = mybir.dt.bfloat16
FP8 = mybir.dt.float8e4
AF = mybir.ActivationFunctionType
ALU = mybir.AluOpType
DR = mybir.MatmulPerfMode.DoubleRow

P = 128
B, S, D, C, A = 4, 2048, 640, 768, 16
H, DH = 8, 80
R = 1024          # rows (tokens) per core
G = 320           # gate hidden
KC_D = 5          # 640 / 128
KC_C = 6          # 768 / 128
NK = 16           # key chunks of 128 over S
SCALE = 1.0 / float(np.sqrt(DH))
EXP_BIAS = -2.0   # exp(scale*z - 2); cancels via normalizer
FLATK = DH * H * R        # elements of one kT shard (bf16)
FLATV = D * R             # elements of one v shard (fp8)

N_CORES = 8
REPLICA_GROUPS = [[0, 1], [2, 3], [4, 5], [6, 7]]

DEBUG = False
SIM_NO_COLLECTIVE = False  # replace AllGather with local DMAs for TimelineSim


def _build_program():
    nc = bacc.Bacc(None, target_bir_lowering=False)

    xT = nc.dram_tensor("xT", [D, R], F32, kind="ExternalInput")        # residual
    xTb = nc.dram_tensor("xTb", [D, R], BF16, kind="ExternalInput")
    xTb8 = nc.dram_tensor("xTb8", [P, 3, 2, R], FP8, kind="ExternalInput")
    auTb = nc.dram_tensor("auTb", [C, A], BF16, kind="ExternalInput")
    wqb = nc.dram_tensor("wqb", [D, D], BF16, kind="ExternalInput")
    wkb = nc.dram_tensor("wkb", [D, D], BF16, kind="ExternalInput")
    wvb8 = nc.dram_tensor("wvb8", [P, 3, 2, D], FP8, kind="ExternalInput")
    wakb = nc.dram_tensor("wakb", [C, D], BF16, kind="ExternalInput")
    wavb = nc.dram_tensor("wavb", [C, D], BF16, kind="ExternalInput")  # x8
    wg1hmb = nc.dram_tensor("wg1hmb", [DH, H, G], BF16, kind="ExternalInput")
    wg2b = nc.dram_tensor("wg2b", [G, D], BF16, kind="ExternalInput")
    wouthmb = nc.dram_tensor("wouthmb", [DH, H, D], BF16, kind="ExternalInput")
    b_g1 = nc.dram_tensor("b_g1", [G], F32, kind="ExternalInput")
    b_g2h = nc.dram_tensor("b_g2h", [D], F32, kind="ExternalInput")  # b_g2/2
    b_out = nc.dram_tensor("b_out", [D], F32, kind="ExternalInput")
    temp = nc.dram_tensor("temperature", [1], F32, kind="ExternalInput")
    outT = nc.dram_tensor("outT", [D, R], F32, kind="ExternalOutput")

    with tile.TileContext(nc) as tc:
        with (
            tc.tile_pool(name="const", bufs=1) as const,
            tc.tile_pool(name="work", bufs=2) as work,
            tc.tile_pool(name="ps_a", bufs=2, space="PSUM") as ps_a,
            tc.tile_pool(name="ps_acc", bufs=1, space="PSUM") as ps_acc,
            tc.tile_pool(name="ps_g", bufs=1, space="PSUM") as ps_g,
            tc.tile_pool(name="dram", bufs=1, space="DRAM") as dram,
        ):
            # ------------- load operands (k path first) -------------
            w_k_bf = const.tile([P, KC_D, D], BF16, name="w_k_bf")
            nc.sync.dma_start(out=w_k_bf[:, 0, :], in_=wkb[0:P, :])
            xT_bf = const.tile([P, KC_D, R], BF16, name="xT_bf", tag="slotx")
            nc.sync.dma_start(out=xT_bf[:, 0, :], in_=xTb[0:P, :])
            nc.sync.dma_start(
                out=w_k_bf[:, 1:, :],
                in_=wkb[P:, :].rearrange("(c p) n -> p c n", p=P),
            )
            nc.sync.dma_start(
                out=xT_bf[:, 1:, :],
                in_=xTb[P:, :].rearrange("(c p) n -> p c n", p=P),
            )
            w_q_bf = const.tile([P, KC_D, D], BF16, name="w_q_bf")
            nc.sync.dma_start(
                out=w_q_bf[:], in_=wqb[:].rearrange("(c p) n -> p c n", p=P)
            )
            exp_b_sb = const.tile([P, 1], F32, name="exp_b_sb")
            nc.vector.memset(exp_b_sb[:], EXP_BIAS)

            # ------------- k local projection + AllGather (split halves) ----
            FK2 = FLATK // 2
            ag_in_k1 = dram.tile([FK2], BF16, name="ag_in_k1")
            ag_in_k2 = dram.tile([FK2], BF16, name="ag_in_k2")
            ag_out_k1 = dram.tile([FLATK], BF16, name="ag_out_k1")
            ag_out_k2 = dram.tile([FLATK], BF16, name="ag_out_k2")
            ag_in_v = dram.tile([FLATV], FP8, name="ag_in_v")
            ag_out_v = dram.tile([2 * FLATV], FP8, name="ag_out_v")
            ag_in_vr = ag_in_v[:].rearrange("(r f) -> r f", f=D)

            kTl_bf = const.tile([P, H, R], BF16, name="kTl_bf", tag="slot16a")

            def k_head(h):
                psk = ps_a.tile([P, R], F32, tag="ps", name=f"psk{h}")
                for qn in range(2):
                    for kc in range(KC_D):
                        nc.tensor.matmul(
                            psk[:DH, qn * 512:(qn + 1) * 512],
                            w_k_bf[:, kc, h * DH:(h + 1) * DH],
                            xT_bf[:, kc, qn * 512:(qn + 1) * 512],
                            start=(kc == 0), stop=(kc == KC_D - 1),
                        )
                nc.vector.tensor_copy(out=kTl_bf[:DH, h, :], in_=psk[:DH, :])

            def k_send(ag_in, ag_out, h0):
                nc.sync.dma_start(
                    out=ag_in[:].rearrange("(h p k) -> p h k", p=DH, k=R),
                    in_=kTl_bf[:DH, h0:h0 + 4, :],
                )
                if SIM_NO_COLLECTIVE:
                    nc.sync.dma_start(out=ag_out[0:FK2], in_=ag_in[:])
                    nc.sync.dma_start(out=ag_out[FK2:FLATK], in_=ag_in[:])
                else:
                    nc.gpsimd.collective_compute(
                        "AllGather",
                        mybir.AluOpType.bypass,
                        replica_groups=REPLICA_GROUPS,
                        ins=[ag_in[:]],
                        outs=[ag_out[:]],
                    )

            for h in range(4):
                k_head(h)
            k_send(ag_in_k1, ag_out_k1, 0)
            for h in range(4, H):
                k_head(h)
            k_send(ag_in_k2, ag_out_k2, 4)

            # ------------- q projection head 0 (window opener) -------------
            qT_bf = const.tile([P, H, R], BF16, name="qT_bf", tag="slot16q")
            nc.gpsimd.memset(qT_bf[64:128, :, :], 0.0)

            def q_head(h):
                psq = ps_acc.tile([P, R], F32, tag="acc", name=f"psq{h}")
                for qn in range(2):
                    for kc in range(KC_D):
                        nc.tensor.matmul(
                            psq[:DH, qn * 512:(qn + 1) * 512],
                            w_q_bf[:, kc, h * DH:(h + 1) * DH],
                            xT_bf[:, kc, qn * 512:(qn + 1) * 512],
                            start=(kc == 0), stop=(kc == KC_D - 1),
                        )
                nc.vector.tensor_copy(out=qT_bf[:DH, h, :], in_=psq[:DH, :])

            q_head(0)
            q_head(1)

            # ---- full kT from AllGather: window-critical DMA chain ----
            kT_bf = const.tile([P, H, S], BF16, name="kT_bf")
            nc.gpsimd.memset(kT_bf[64:128, :, :], 0.0)
            for hg, ag_out in ((0, ag_out_k1), (4, ag_out_k2)):
                for s in range(2):
                    nc.sync.dma_start(
                        out=kT_bf[:DH, hg:hg + 4, s * R:(s + 1) * R],
                        in_=ag_out[s * FK2:(s + 1) * FK2].rearrange(
                            "(h p k) -> p h k", p=DH, k=R
                        ),
                    )
            # fp8 v-path operands (deferred so k-path DMAs go first)
            xT_8 = const.tile([P, 3, 2, R], FP8, name="xT_8")
            nc.sync.dma_start(out=xT_8[:], in_=xTb8[:])
            w_v_8 = const.tile([P, 3, 2, D], FP8, name="w_v_8")
            nc.sync.dma_start(out=w_v_8[:], in_=wvb8[:])

            # ------------- v local projection (fp8 DoubleRow) + AllGather ----
            for vb in range(16):  # DoubleRow needs dst base 0, M<=64
                psv = ps_a.tile([P, R], F32, tag="ps", name=f"psv{vb}")
                for ns, w in ((0, 512), (512, 128)):
                    for pc in range(3):
                        nc.tensor.matmul(
                            psv[:64, ns:ns + w],
                            xT_8[:, pc, :, vb * 64:(vb + 1) * 64],
                            w_v_8[:, pc, :, ns:ns + w],
                            start=(pc == 0), stop=(pc == 2),
                            perf_mode=DR,
                        )
                v_sb = work.tile([64, D], FP8, tag="probsT", bufs=4,
                                 name=f"v_sb{vb}")
                nc.vector.tensor_copy(out=v_sb[:], in_=psv[:64, :D])
                nc.sync.dma_start(
                    out=ag_in_vr[vb * 64:(vb + 1) * 64, :], in_=v_sb[:]
                )

            if SIM_NO_COLLECTIVE:
                nc.sync.dma_start(out=ag_out_v[0:FLATV], in_=ag_in_v[:])
                nc.sync.dma_start(out=ag_out_v[FLATV:2 * FLATV], in_=ag_in_v[:])
            else:
                nc.gpsimd.collective_compute(
                    "AllGather",
                    mybir.AluOpType.bypass,
                    replica_groups=REPLICA_GROUPS,
                    ins=[ag_in_v[:]],
                    outs=[ag_out_v[:]],
                )

            # last dim padded 81->82 so the kc-pair step (8*82=656) is %16==0
            # as DoubleRow's stationary AP requires.
            v_aug = const.tile([P, NK, H, 82], FP8, name="v_aug", tag="slot20")
            nc.gpsimd.memset(v_aug[:, :, :, DH:DH + 1], 1.0)
            for s in range(2):
                vsh = ag_out_v[s * FLATV:(s + 1) * FLATV].rearrange(
                    "(r f) -> r f", f=D
                )
                for rc in range(8):
                    nc.sync.dma_start(
                        out=v_aug[:, s * 8 + rc, :, 0:DH],
                        in_=vsh[rc * P:(rc + 1) * P, :].rearrange(
                            "p (h d) -> p h d", d=DH
                        ),
                    )

            # AU operands (deferred loads)
            w_ak_bf = const.tile([P, KC_C, D], BF16, name="w_ak_bf")
            nc.sync.dma_start(
                out=w_ak_bf[:], in_=wakb[:].rearrange("(c p) n -> p c n", p=P)
            )
            w_av_bf = const.tile([P, KC_C, D], BF16, name="w_av_bf")
            nc.sync.dma_start(
                out=w_av_bf[:], in_=wavb[:].rearrange("(c p) n -> p c n", p=P)
            )
            auT_bf = const.tile([P, KC_C, A], BF16, name="auT_bf")
            nc.sync.dma_start(
                out=auT_bf[:], in_=auTb[:].rearrange("(c p) n -> p c n", p=P)
            )
            t_sb = const.tile([P, 1], F32, name="t_sb")
            nc.sync.dma_start(out=t_sb[:], in_=temp[:].to_broadcast((P, 1)))
            alpha_s = const.tile([P, 1], F32, name="alpha_s")
            nc.vector.tensor_scalar_mul(alpha_s[:], t_sb[:], SCALE)

            # ------------- AU cross-attention constants -------------
            au_kT_s = const.tile([P, H, A], BF16, name="au_kT_s")
            nc.gpsimd.memset(au_kT_s[64:128, :, :], 0.0)
            for h in range(H):
                psak = ps_g.tile([P, R], F32, tag="psg", name=f"psak{h}")
                for kc in range(KC_C):
                    nc.tensor.matmul(
                        psak[:DH, 0:A],
                        w_ak_bf[:, kc, h * DH:(h + 1) * DH],
                        auT_bf[:, kc, :],
                        start=(kc == 0), stop=(kc == KC_C - 1),
                    )
                nc.vector.tensor_scalar_mul(
                    au_kT_s[:DH, h, :], psak[:DH, 0:A], alpha_s[:DH]
                )

            au_v_aug = const.tile([P, H, DH + 1], BF16, name="au_v_aug")
            nc.gpsimd.memset(au_v_aug[:], 0.0)
            nc.gpsimd.memset(au_v_aug[:A, :, DH:DH + 1], 1.0)
            psav = ps_g.tile([P, R], F32, tag="psg", name="psav")
            for ns, w in ((0, 512), (512, 128)):
                for kc in range(KC_C):
                    nc.tensor.matmul(
                        psav[:A, ns:ns + w],
                        auT_bf[:, kc, :],
                        w_av_bf[:, kc, ns:ns + w],
                        start=(kc == 0), stop=(kc == KC_C - 1),
                    )
            nc.vector.tensor_copy(
                out=au_v_aug[:A, :, 0:DH],
                in_=psav[:A, 0:D].rearrange("p (h d) -> p h d", d=DH),
            )

            # persistent AU probs tile: zeroed once, exps rewrite rows 0:16
            au_pT = const.tile([P, R], BF16, name="au_pT")
            nc.gpsimd.memset(au_pT[:, :], 0.0)

            # ------------- main self-attention -------------
            dram_hs_sums = dram.tile([H, R], BF16, name="dram_hs_sums")
            hs_keep = []

            def attn_head(h):
                pshs = ps_acc.tile([P, R], F32, tag="acc", name=f"pshs{h}")
                for c in range(NK // 2):
                    pt = work.tile([P, 2, R], FP8, tag="probsT", bufs=4,
                                   name=f"pt{h}_{c}")
                    for j in range(2):
                        pslog = ps_a.tile([P, R], F32, tag="ps",
                                          name=f"pslog{h}_{c}_{j}")
                        kc = 2 * c + j
                        with tc.high_priority():
                            for qn in range(2):
                                nc.tensor.matmul(
                                    pslog[:, qn * 512:(qn + 1) * 512],
                                    kT_bf[:, h, kc * P:(kc + 1) * P],
                                    qT_bf[:, h, qn * 512:(qn + 1) * 512],
                                    start=True, stop=True,
                                )
                        nc.scalar.activation(out=pt[:, j, :], in_=pslog[:],
                                             func=AF.Exp, scale=SCALE,
                                             bias=exp_b_sb[:, 0:1])
                    with tc.high_priority():
                        for qn in range(2):
                            nc.tensor.matmul(
                                pshs[:DH + 1, qn * 512:(qn + 1) * 512],
                                v_aug[:, 2 * c:2 * c + 2, h, 0:DH + 1],
                                pt[:, :, qn * 512:(qn + 1) * 512],
                                start=(c == 0),
                                stop=(c == NK // 2 - 1),
                                perf_mode=DR,
                            )
                hs_st = work.tile([P, R], BF16, tag="hs_keep", bufs=8,
                                  name=f"hs_st{h}")
                nc.vector.tensor_copy(out=hs_st[:DH + 1, :], in_=pshs[:DH + 1, :])
                if h < 6:
                    nc.sync.dma_start(out=dram_hs_sums[h], in_=hs_st[DH:DH + 1, :])
                hs_keep.append(hs_st)

            dram_rec_row = dram.tile([H, R], BF16, name="dram_rec_row")

            def row_recip_mul(sums_row, dst, src, h, name):
                """dst = src * (1/sums_row); recip on the [1,R] sums row, then
                a 2KB DRAM bounce for the partition broadcast. The sums row
                lives on partition 80, so it reaches partition 0 by DMA
                (engines cannot shift partitions)."""
                r16 = work.tile([1, R], BF16, tag="rr16", bufs=1,
                                name=f"{name}_16")
                nc.sync.dma_start(out=r16[:], in_=sums_row)
                rb = work.tile([1, R], BF16, tag="rrb", bufs=1, name=f"{name}_b")
                for qn in range(2):
                    sl = np.s_[:, qn * 512:(qn + 1) * 512]
                    rf = work.tile([1, 512], F32, tag="rrf", bufs=1,
                                   name=f"{name}_f{qn}")
                    nc.vector.tensor_copy(out=rf[:], in_=r16[sl])
                    ro = work.tile([1, 512], F32, tag="rro", bufs=1,
                                   name=f"{name}_o{qn}")
                    nc.vector.reciprocal_approx_fast(ro[:], rf[:])
                    nc.vector.tensor_copy(out=rb[sl], in_=ro[:])
                nc.sync.dma_start(out=dram_rec_row[h, :], in_=rb[0:1, :])
                bc = work.tile([DH, R], BF16, tag="bc", bufs=1, name=f"{name}_bc")
                nc.sync.dma_start(
                    out=bc[:], in_=dram_rec_row[h:h + 1, :].to_broadcast((DH, R))
                )
                nc.vector.tensor_mul(dst, src, bc[:])

            dram_au = dram.tile([H, DH + 1, R], BF16, name="dram_au")

            def au_head(h):
                psal = ps_g.tile([P, R], F32, tag="psg", name=f"psal{h}")
                for qn in range(2):
                    nc.tensor.matmul(
                        psal[:A, qn * 512:(qn + 1) * 512],
                        au_kT_s[:, h, :],
                        qT_bf[:, h, qn * 512:(qn + 1) * 512],
                        start=True, stop=True,
                    )
                nc.scalar.activation(out=au_pT[:A, :], in_=psal[:A, :],
                                     func=AF.Exp)
                psau = ps_g.tile([P, R], F32, tag="psg", name=f"psau{h}")
                for qn in range(2):
                    nc.tensor.matmul(
                        psau[:DH + 1, qn * 512:(qn + 1) * 512],
                        au_v_aug[:, h, :],
                        au_pT[:, qn * 512:(qn + 1) * 512],
                        start=True, stop=True,
                    )
                au_st = work.tile([P, R], BF16, tag="evac", bufs=1,
                                  name=f"au_st{h}")
                nc.vector.tensor_copy(out=au_st[:DH + 1, :], in_=psau[:DH + 1, :])
                nc.sync.dma_start(out=dram_au[h], in_=au_st[:DH + 1, :])

            # reciprocal chain: per-(head,query) sums -> 1/sum (bf16) in DRAM
            def recip_chain(sums_src, name, dma_eng):
                rc_in = work.tile([P, 64], BF16, tag="rc", bufs=1, name=f"{name}_in")
                for h in range(H):
                    dma_eng.dma_start(
                        out=rc_in[h * 16:(h + 1) * 16, :],
                        in_=sums_src(h),
                    )
                rc_f = work.tile([P, 64], F32, tag="rcf", bufs=1, name=f"{name}_f")
                nc.vector.tensor_copy(out=rc_f[:], in_=rc_in[:])
                rc_s = work.tile([P, 64], F32, tag="rcs", bufs=1, name=f"{name}_s")
                rc_o = work.tile([P, 64], F32, tag="rco", bufs=1, name=f"{name}_o")
                nc.vector.reciprocal_approx_accurate(rc_o[:], rc_f[:], rc_s[:])
                rc_b = work.tile([P, 64], BF16, tag="rcb", bufs=1, name=f"{name}_b")
                nc.vector.tensor_copy(out=rc_b[:], in_=rc_o[:])
                drec = dram.tile([H, R], BF16, name=f"{name}_dr")
                dma_eng.dma_start(
                    out=drec[:].rearrange("h (a j) -> (h a) j", j=64), in_=rc_b[:]
                )
                return drec

            au_rec_ref = []

            def au_finish():
                dram_au_rec = recip_chain(
                    lambda h: dram_au[h, DH, :].rearrange("(a j) -> a j", j=64),
                    "aurec", nc.sync,
                )
                nc.gpsimd.memset(au_hsT[64:128, :, :], 0.0)
                for h in range(H):
                    bc = work.tile([DH, R], BF16, tag="bc", bufs=1, name=f"aubc{h}")
                    nc.sync.dma_start(
                        out=bc[:], in_=dram_au_rec[h:h + 1, :].to_broadcast((DH, R))
                    )
                    au_ld = work.tile([DH, R], BF16, tag="evac", bufs=1,
                                      name=f"auld{h}")
                    nc.sync.dma_start(out=au_ld[:], in_=dram_au[h, 0:DH, :])
                    nc.vector.tensor_mul(au_hsT[:DH, h, :], au_ld[:], bc[:])
                au_rec_ref.append(dram_au_rec)

            au_hsT = const.tile([P, H, R], BF16, name="au_hsT", tag="slot16a")

            # ---- AU heads + q heads interleave under the early window ----
            au_head(0)
            au_head(1)

            attn_head(0)
            q_head(2)
            q_head(3)
            au_head(2)
            au_head(3)
            attn_head(1)
            q_head(4)
            q_head(5)
            au_head(4)
            au_head(5)
            attn_head(2)
            q_head(6)
            q_head(7)
            au_head(6)
            au_head(7)
            au_finish()

            # late loads: gate + out-proj weights (DMA slack mid-window)
            w_g1_hm = const.tile([P, H, G], BF16, name="w_g1_hm")
            nc.gpsimd.memset(w_g1_hm[64:128, :, :], 0.0)
            nc.sync.dma_start(out=w_g1_hm[:DH, :, :], in_=wg1hmb[:])
            w_g2_bf = const.tile([P, 3, D], BF16, name="w_g2_bf")
            nc.sync.dma_start(
                out=w_g2_bf[:, 0:2, :],
                in_=wg2b[0:256, :].rearrange("(c p) n -> p c n", p=P),
            )
            nc.sync.dma_start(out=w_g2_bf[:64, 2, :], in_=wg2b[256:320, :])
            nc.gpsimd.memset(w_g2_bf[64:128, 2, :], 0.0)
            w_out_hm = const.tile([P, H, D], BF16, name="w_out_hm")
            nc.gpsimd.memset(w_out_hm[64:128, :, :], 0.0)
            nc.sync.dma_start(out=w_out_hm[:DH, :, :], in_=wouthmb[:])
            b_g1_sb = const.tile([P, 3], F32, name="b_g1_sb")
            nc.vector.memset(b_g1_sb[:], 0.0)
            nc.sync.dma_start(
                out=b_g1_sb[:, 0:2], in_=b_g1[0:256].rearrange("(c p) -> p c", p=P)
            )
            nc.sync.dma_start(out=b_g1_sb[:64, 2:3], in_=b_g1[256:320][:, None])
            b_g2_hm = const.tile([P, H], F32, name="b_g2_hm")  # holds b_g2/2
            nc.vector.memset(b_g2_hm[:], 0.0)
            nc.sync.dma_start(
                out=b_g2_hm[:DH, :], in_=b_g2h[:].rearrange("(h p) -> p h", p=DH)
            )
            b_out_sb = const.tile([P, KC_D], F32, name="b_out_sb")
            nc.sync.dma_start(
                out=b_out_sb[:], in_=b_out[:].rearrange("(c p) -> p c", p=P)
            )

            # residual+bias rows preloaded so out-proj finishes don't wait DMA
            rx_t = {}
            for mo in range(KC_D):
                rx = work.tile([P, R], F32, tag="rx", bufs=3, name=f"rx{mo}")
                nc.sync.dma_start(out=rx[:], in_=xT[mo * P:(mo + 1) * P, :])
                nc.vector.tensor_scalar_add(rx[:], rx[:], b_out_sb[:, mo:mo + 1])
                rx_t[mo] = rx

            # ---- gate MLP (emitted here; runs under attn3-5 exp windows) ----
            siluT = const.tile([P, 3, R], BF16, name="siluT", tag="slot16q_silu")
            nc.gpsimd.memset(siluT[64:128, 2, :], 0.0)
            for mo, rows in ((0, 128), (1, 128), (2, 64)):
                psl1 = ps_g.tile([P, R], F32, tag="psg", name=f"psl1{mo}")
                for qn in range(2):
                    for h in range(H):
                        nc.tensor.matmul(
                            psl1[:rows, qn * 512:(qn + 1) * 512],
                            w_g1_hm[:, h, mo * P:mo * P + rows],
                            au_hsT[:, h, qn * 512:(qn + 1) * 512],
                            start=(h == 0), stop=(h == H - 1),
                        )
                nc.scalar.activation(
                    out=siluT[:rows, mo, :], in_=psl1[:rows, :],
                    func=AF.Silu, scale=0.125, bias=b_g1_sb[:rows, mo:mo + 1],
                )

            fusedA = const.tile([P, 6, R], BF16, name="fusedA", tag="slotx")
            nc.gpsimd.memset(fusedA[64:128, :, :], 0.0)
            fusedB6 = const.tile([P, R], BF16, name="fusedB6")
            nc.gpsimd.memset(fusedB6[64:128, :], 0.0)
            fusedB7 = const.tile([P, R], BF16, name="fusedB7")
            nc.gpsimd.memset(fusedB7[64:128, :], 0.0)

            def fused_sl(h):
                if h < 6:
                    return fusedA[:, h, :]
                return fusedB6[:, :] if h == 6 else fusedB7[:, :]

            def gate_head(h):
                # tanh(psg/2 + b_g2/2); gate*au_hs = (1+t)*au_hs/2
                psg = ps_g.tile([P, R], F32, tag="psg", name=f"psgate{h}")
                for qn in range(2):
                    for kc in range(3):
                        nc.tensor.matmul(
                            psg[:DH, qn * 512:(qn + 1) * 512],
                            w_g2_bf[:, kc, h * DH:(h + 1) * DH],
                            siluT[:, kc, qn * 512:(qn + 1) * 512],
                            start=(kc == 0), stop=(kc == 2),
                        )
                gateT = work.tile([DH, R], BF16, tag="gateT", bufs=1,
                                  name=f"gateT{h}")
                nc.scalar.activation(
                    out=gateT[:], in_=psg[:DH, :],
                    func=AF.Tanh, scale=0.5, bias=b_g2_hm[:DH, h:h + 1],
                )
                # fused = (gateT + 1) * au_hsT  (au_hsT carries x8 => x16 net)
                nc.vector.scalar_tensor_tensor(
                    out=fused_sl(h)[:DH, :], in0=gateT[:], scalar=1.0,
                    in1=au_hsT[:DH, h, :], op0=ALU.add, op1=ALU.mult,
                )

            for h in range(H):
                gate_head(h)

            attn_head(3)
            attn_head(4)
            attn_head(5)

            # ---- hs recip chain A: heads 0..5; fused(h) += hs*rec ----
            rcA_in = work.tile([P, 64], BF16, tag="rc", bufs=1, name="rcA_in")
            for h in range(6):
                nc.sync.dma_start(
                    out=rcA_in[h * 16:(h + 1) * 16, :],
                    in_=dram_hs_sums[h, :].rearrange("(a j) -> a j", j=64),
                )
            rcA_f = work.tile([P, 64], F32, tag="rcf", bufs=1, name="rcA_f")
            nc.vector.memset(rcA_f[96:, :], 1.0)
            nc.vector.tensor_copy(out=rcA_f[:96, :], in_=rcA_in[:96, :])
            rcA_s = work.tile([P, 64], F32, tag="rcs", bufs=1, name="rcA_s")
            rcA_o = work.tile([P, 64], F32, tag="rco", bufs=1, name="rcA_o")
            nc.vector.reciprocal_approx_accurate(rcA_o[:], rcA_f[:], rcA_s[:])
            rcA_b = work.tile([P, 64], BF16, tag="rcb", bufs=1, name="rcA_b")
            nc.vector.tensor_copy(out=rcA_b[:96, :], in_=rcA_o[:96, :])
            dram_hs_rec = dram.tile([H, R], BF16, name="hsrec_dr")
            nc.sync.dma_start(
                out=dram_hs_rec[0:6, :].rearrange("h (a j) -> (h a) j", j=64),
                in_=rcA_b[:96, :],
            )
            for h in range(6):
                bch = work.tile([DH, R], BF16, tag="bc", bufs=1, name=f"hsbc{h}")
                nc.sync.dma_start(
                    out=bch[:], in_=dram_hs_rec[h:h + 1, :].to_broadcast((DH, R))
                )
                hs_st = hs_keep[h]
                nc.vector.tensor_mul(hs_st[:DH, :], hs_st[:DH, :], bch[:])
                nc.vector.tensor_add(
                    fused_sl(h)[:DH, :], fused_sl(h)[:DH, :], hs_st[:DH, :]
                )

            attn_head(6)

            # ---- per-head tail recip for h6/h7 ----
            def recip_tail(h):
                # fast SBUF-only recip: sums row is hs_st row 80
                row_recip_mul(
                    hs_keep[h][DH:DH + 1, :], hs_keep[h][:DH, :],
                    hs_keep[h][:DH, :], h, f"rt{h}",
                )
                nc.vector.tensor_add(
                    fused_sl(h)[:DH, :], fused_sl(h)[:DH, :], hs_keep[h][:DH, :]
                )

            recip_tail(6)

            # ------------- output projection + residual -------------
            pso_t = {}

            PSO_POOL = {0: (ps_g, "psg"), 1: (ps_a, "ps"), 2: (ps_acc, "acc"),
                        3: (ps_a, "ps"), 4: (ps_g, "psg")}

            def out_partial(mo, h0, h1):
                if mo not in pso_t:
                    pool, tg = PSO_POOL[mo]
                    pso_t[mo] = pool.tile([P, R], F32, tag=tg, name=f"pso{mo}")
                t = pso_t[mo]
                for qn in range(2):
                    for h in range(h0, h1):
                        nc.tensor.matmul(
                            t[:, qn * 512:(qn + 1) * 512],
                            w_out_hm[:, h, mo * P:(mo + 1) * P],
                            fused_sl(h)[:, qn * 512:(qn + 1) * 512],
                            start=(h == 0), stop=(h == H - 1),
                        )

            def out_finish(mo):
                # out = pso/16 + (residual + b_out), in place on the rx tile
                rx = rx_t[mo]
                nc.vector.scalar_tensor_tensor(
                    out=rx[:], in0=pso_t[mo][:], scalar=0.0625,
                    in1=rx[:], op0=ALU.mult, op1=ALU.add,
                )
                nc.sync.dma_start(out=outT[mo * P:(mo + 1) * P, :], in_=rx[:])

            attn_head(7)
            # h0-6 partials for mo 0,1 run under attn7's exp window
            out_partial(0, 0, 7)
            out_partial(1, 0, 7)

            recip_tail(7)

            out_partial(0, 7, H)
            out_finish(0)
            for mo in range(2, KC_D):
                out_partial(mo, 0, 7)
                out_partial(mo - 1, 7, H)
                out_finish(mo - 1)
            out_partial(KC_D - 1, 7, H)
            out_finish(KC_D - 1)

    nc.finalize()
    return nc


_NC_CACHE = []


def get_program():
    if not _NC_CACHE:
        _NC_CACHE.append(_build_program())
    return _NC_CACHE[0]


def _bf(x):
    import ml_dtypes
    return np.ascontiguousarray(x.astype(ml_dtypes.bfloat16))


def _f8(x):
    import ml_dtypes
    return np.ascontiguousarray(x.astype(ml_dtypes.float8_e4m3))


def _pair_chunks(w, rows, cols, scale):
    """[rows<=768, cols] -> fp8 [128, 3, 2, cols] over d-chunk pairs."""
    out = np.zeros((P, 3, 2, cols), np.float32)
    for pc in range(3):
        for i in range(2):
            r0 = (2 * pc + i) * P
            r1 = min(r0 + P, rows)
            if r0 < rows:
                out[: r1 - r0, pc, i, :] = w[r0:r1, :]
    return _f8(out * scale)


def kernel(**inputs):
    f = lambda k: np.ascontiguousarray(np.asarray(inputs[k], dtype=np.float32))
    hidden = f("hidden_states")          # [4, 2048, 640]
    au = f("au_embedding")               # [4, 16, 768]
    w_g1 = f("w_g1")                     # [640, 320]
    w_out_w = f("w_out")                 # [640, 640]
    shared = {
        "wqb": _bf(f("w_q")),
        "wkb": _bf(f("w_k")),
        "wvb8": _pair_chunks(f("w_v"), D, D, 16.0),
        "wakb": _bf(f("w_ak")),
        "wavb": _bf(f("w_av") * 8.0),
        "wg1hmb": _bf(w_g1.reshape(H, DH, G).transpose(1, 0, 2)),
        "wg2b": _bf(f("w_g2")),
        "wouthmb": _bf(w_out_w.reshape(H, DH, D).transpose(1, 0, 2)),
        "b_g1": f("b_g1"),
        "b_g2h": f("b_g2") * 0.5,
        "b_out": f("b_out"),
        "temperature": f("temperature"),
    }
    in_maps = []
    for c in range(N_CORES):
        b, half = divmod(c, 2)
        m = dict(shared)
        xt = np.ascontiguousarray(hidden[b, half * R:(half + 1) * R, :].T)
        m["xT"] = xt
        m["xTb"] = _bf(xt)
        m["xTb8"] = _pair_chunks(xt, D, R, 1.0)
        m["auTb"] = _bf(np.ascontiguousarray(au[b].T))
        in_maps.append(m)

    nc = get_program()
    try:
        res = run_bass_kernel_spmd(nc, in_maps, core_ids=list(range(N_CORES)))
    except Exception:
        # transient device wedge (NRT_EXEC_UNIT_UNRECOVERABLE) — retry once
        import time as _time
        _time.sleep(10)
        res = run_bass_kernel_spmd(nc, in_maps, core_ids=list(range(N_CORES)))

    out = np.empty((B, S, D), dtype=np.float32)
    for c in range(N_CORES):
        b, half = divmod(c, 2)
        out[b, half * R:(half + 1) * R, :] = res.results[c]["outT"].T
    return out


# revision 46
# speedup vs baseline: 1.0306x; 1.0306x over previous
"""Trainium2 Bass kernel for nn_AUAttnProcessor (self-attn + AU cross-attn + gated fusion).

Sharding: 8 cores = 4 batches x 2 sequence-halves. Each core computes its
1024 query rows end-to-end in a feature-major ("transposed", [D, tokens])
dataflow. k/v are computed locally per half and AllGathered within the
2-core batch pair.

Numerics (validated vs reference on CPU, rel ~2.6e-3):
- bf16: x, w_q/w_k, q/k, logits, whole AU attention (w_av host-scaled x8
  so au_hs matches hs scale), hs, fused, w_out, out projection.
- fp8e4m3: v path (x8 input copy, w_v host-scaled x16, v, v_aug, its
  AllGather payload), main-attention probs.
- Main softmax: exp(SCALE*logits - 2) -- the -2 keeps probs under fp8 max
  (z reaches ~6.7); the factor e^-2 cancels exactly through the ones-column
  normalizer. AU probs stay bf16 (temperature-5 logits reach e^26).
- PV runs fp8 DoubleRow: probs tiles hold kc pairs [128, 2, R] so one
  matmul contracts 256 keys at 0.5 cycles/col.
- gate: sigmoid(x) = 0.5*(1+tanh(x/2)) -- Tanh shares the Exp activation
  table, so the only table switches are around the 3 Silu instructions.
  fused = (1+tanh)*au_hs8 + hs16 via one scalar_tensor_tensor per head.
- out = fused(16x) @ w_out -> evac (psum * 1/16 + residual + b_out).

Schedule: window is ACT(exp)-paced (~157us of activation work). k proj ->
AGk -> q h0 -> attention starts on kT slot 0 while everything else (q h1-7,
v proj, AU, gate, out-proj partials) fills PE slack under the exp window.
"""

import numpy as np

import concourse.bacc as bacc
import concourse.bass as bass
import concourse.tile as tile
from concourse import mybir
from concourse.bass_utils import run_bass_kernel_spmd

F32 = mybir.dt.float32
BF16 = mybir.dt.bfloat16
FP8 = mybir.dt.float8e4
AF = mybir.ActivationFunctionType
ALU = mybir.AluOpType
DR = mybir.MatmulPerfMode.DoubleRow

P = 128
B, S, D, C, A = 4, 2048, 640, 768, 16
H, DH = 8, 80
R = 1024          # rows (tokens) per core
G = 320           # gate hidden
KC_D = 5          # 640 / 128
KC_C = 6          # 768 / 128
NK = 16           # key chunks of 128 over S
SCALE = 1.0 / float(np.sqrt(DH))
EXP_BIAS = -2.0   # exp(scale*z - 2); cancels via normalizer
FLATK = DH * H * R        # elements of one kT shard (bf16)
FLATV = D * R             # elements of one v shard (fp8)

N_CORES = 8
REPLICA_GROUPS = [[0, 1], [2, 3], [4, 5], [6, 7]]

DEBUG = False
SIM_NO_COLLECTIVE = False  # replace AllGather with local DMAs for TimelineSim


def _build_program():
    nc = bacc.Bacc(None, target_bir_lowering=False)

    xT = nc.dram_tensor("xT", [D, R], F32, kind="ExternalInput")        # residual
    xTb = nc.dram_tensor("xTb", [D, R], BF16, kind="ExternalInput")
    xTb8 = nc.dram_tensor("xTb8", [P, 3, 2, R], FP8, kind="ExternalInput")
    auTb = nc.dram_tensor("auTb", [C, A], BF16, kind="ExternalInput")
    wqb = nc.dram_tensor("wqb", [D, D], BF16, kind="ExternalInput")
    wkb = nc.dram_tensor("wkb", [D, D], BF16, kind="ExternalInput")
    wvb8 = nc.dram_tensor("wvb8", [P, 3, 2, D], FP8, kind="ExternalInput")
    wakb = nc.dram_tensor("wakb", [C, D], BF16, kind="ExternalInput")
    wavb = nc.dram_tensor("wavb", [C, D], BF16, kind="ExternalInput")  # x8
    wg1hmb = nc.dram_tensor("wg1hmb", [DH, H, G], BF16, kind="ExternalInput")
    wg2b = nc.dram_tensor("wg2b", [G, D], BF16, kind="ExternalInput")
    wouthmb = nc.dram_tensor("wouthmb", [DH, H, D], BF16, kind="ExternalInput")
    b_g1 = nc.dram_tensor("b_g1", [G], F32, kind="ExternalInput")
    b_g2h = nc.dram_tensor("b_g2h", [D], F32, kind="ExternalInput")  # b_g2/2
    b_out = nc.dram_tensor("b_out", [D], F32, kind="ExternalInput")
    temp = nc.dram_tensor("temperature", [1], F32, kind="ExternalInput")
    outT = nc.dram_tensor("outT", [D, R], F32, kind="ExternalOutput")

    with tile.TileContext(nc) as tc:
        with (
            tc.tile_pool(name="const", bufs=1) as const,
            tc.tile_pool(name="work", bufs=2) as work,
            tc.tile_pool(name="ps_a", bufs=2, space="PSUM") as ps_a,
            tc.tile_pool(name="ps_acc", bufs=1, space="PSUM") as ps_acc,
            tc.tile_pool(name="ps_g", bufs=1, space="PSUM") as ps_g,
            tc.tile_pool(name="dram", bufs=1, space="DRAM") as dram,
        ):
            # ------------- load operands (k path first) -------------
            w_k_bf = const.tile([P, KC_D, D], BF16, name="w_k_bf")
            nc.sync.dma_start(out=w_k_bf[:, 0, :], in_=wkb[0:P, :])
            xT_bf = const.tile([P, KC_D, R], BF16, name="xT_bf", tag="slotx")
            nc.sync.dma_start(out=xT_bf[:, 0, :], in_=xTb[0:P, :])
            nc.sync.dma_start(
                out=w_k_bf[:, 1:, :],
                in_=wkb[P:, :].rearrange("(c p) n -> p c n", p=P),
            )
            nc.sync.dma_start(
                out=xT_bf[:, 1:, :],
                in_=xTb[P:, :].rearrange("(c p) n -> p c n", p=P),
            )
            w_q_bf = const.tile([P, KC_D, D], BF16, name="w_q_bf")
            nc.sync.dma_start(
                out=w_q_bf[:], in_=wqb[:].rearrange("(c p) n -> p c n", p=P)
            )
            exp_b_sb = const.tile([P, 1], F32, name="exp_b_sb")
            nc.vector.memset(exp_b_sb[:], EXP_BIAS)

            # ------------- k local projection + AllGather (split halves) ----
            FK2 = FLATK // 2
            ag_in_k1 = dram.tile([FK2], BF16, name="ag_in_k1")
            ag_in_k2 = dram.tile([FK2], BF16, name="ag_in_k2")
            ag_out_k1 = dram.tile([FLATK], BF16, name="ag_out_k1")
            ag_out_k2 = dram.tile([FLATK], BF16, name="ag_out_k2")
            ag_in_v = dram.tile([FLATV], FP8, name="ag_in_v")
            ag_out_v = dram.tile([2 * FLATV], FP8, name="ag_out_v")
            ag_in_vr = ag_in_v[:].rearrange("(r f) -> r f", f=D)

            kTl_bf = const.tile([P, H, R], BF16, name="kTl_bf", tag="slot16a")

            def k_head(h):
                psk = ps_a.tile([P, R], F32, tag="ps", name=f"psk{h}")
                for qn in range(2):
                    for kc in range(KC_D):
                        nc.tensor.matmul(
                            psk[:DH, qn * 512:(qn + 1) * 512],
                            w_k_bf[:, kc, h * DH:(h + 1) * DH],
                            xT_bf[:, kc, qn * 512:(qn + 1) * 512],
                            start=(kc == 0), stop=(kc == KC_D - 1),
                        )
                nc.vector.tensor_copy(out=kTl_bf[:DH, h, :], in_=psk[:DH, :])

            def k_head_qn(h, qn):
                # intra-window variant: single qn-half on the ps_g pool
                psk = ps_g.tile([P, R], F32, tag="psg", name=f"pskq{h}_{qn}")
                for kc in range(KC_D):
                    nc.tensor.matmul(
                        psk[:DH, 0:512],
                        w_k_bf[:, kc, h * DH:(h + 1) * DH],
                        xT_bf[:, kc, qn * 512:(qn + 1) * 512],
                        start=(kc == 0), stop=(kc == KC_D - 1),
                    )
                nc.vector.tensor_copy(
                    out=kTl_bf[:DH, h, qn * 512:(qn + 1) * 512],
                    in_=psk[:DH, 0:512],
                )

            def k_send(ag_in, ag_out, h0):
                nc.sync.dma_start(
                    out=ag_in[:].rearrange("(h p k) -> p h k", p=DH, k=R),
                    in_=kTl_bf[:DH, h0:h0 + 4, :],
                )
                if SIM_NO_COLLECTIVE:
                    nc.sync.dma_start(out=ag_out[0:FK2], in_=ag_in[:])
                    nc.sync.dma_start(out=ag_out[FK2:FLATK], in_=ag_in[:])
                else:
                    nc.gpsimd.collective_compute(
                        "AllGather",
                        mybir.AluOpType.bypass,
                        replica_groups=REPLICA_GROUPS,
                        ins=[ag_in[:]],
                        outs=[ag_out[:]],
                    )

            for h in range(4):
                k_head(h)
            k_send(ag_in_k1, ag_out_k1, 0)

            # ------------- q projection head 0 (window opener) -------------
            qT_bf = const.tile([P, H, R], BF16, name="qT_bf", tag="slot16q")
            nc.gpsimd.memset(qT_bf[64:128, :, :], 0.0)

            def q_head(h):
                psq = ps_acc.tile([P, R], F32, tag="acc", name=f"psq{h}")
                for qn in range(2):
                    for kc in range(KC_D):
                        nc.tensor.matmul(
                            psq[:DH, qn * 512:(qn + 1) * 512],
                            w_q_bf[:, kc, h * DH:(h + 1) * DH],
                            xT_bf[:, kc, qn * 512:(qn + 1) * 512],
                            start=(kc == 0), stop=(kc == KC_D - 1),
                        )
                nc.vector.tensor_copy(out=qT_bf[:DH, h, :], in_=psq[:DH, :])

            def q_head_qn(h, qn):
                # intra-window variant: single qn-half on the ps_g pool
                psq = ps_g.tile([P, R], F32, tag="psg", name=f"psqq{h}_{qn}")
                for kc in range(KC_D):
                    nc.tensor.matmul(
                        psq[:DH, 0:512],
                        w_q_bf[:, kc, h * DH:(h + 1) * DH],
                        xT_bf[:, kc, qn * 512:(qn + 1) * 512],
                        start=(kc == 0), stop=(kc == KC_D - 1),
                    )
                nc.vector.tensor_copy(
                    out=qT_bf[:DH, h, qn * 512:(qn + 1) * 512],
                    in_=psq[:DH, 0:512],
                )

            q_head(0)

            # ---- full kT from AllGather: window-critical DMA chain ----
            kT_bf = const.tile([P, H, S], BF16, name="kT_bf")
            nc.gpsimd.memset(kT_bf[64:128, :, :], 0.0)

            def kt_fill(hg, ag_out):
                for s in range(2):
                    nc.sync.dma_start(
                        out=kT_bf[:DH, hg:hg + 4, s * R:(s + 1) * R],
                        in_=ag_out[s * FK2:(s + 1) * FK2].rearrange(
                            "(h p k) -> p h k", p=DH, k=R
                        ),
                    )

            kt_fill(0, ag_out_k1)
            # fp8 v-path operands (deferred so k-path DMAs go first)
            xT_8 = const.tile([P, 3, 2, R], FP8, name="xT_8")
            nc.sync.dma_start(out=xT_8[:], in_=xTb8[:])
            w_v_8 = const.tile([P, 3, 2, D], FP8, name="w_v_8")
            nc.sync.dma_start(out=w_v_8[:], in_=wvb8[:])

            # ------------- v local projection (fp8 DoubleRow) + AllGather ----
            for vb in range(16):  # DoubleRow needs dst base 0, M<=64
                psv = ps_a.tile([P, R], F32, tag="ps", name=f"psv{vb}")
                for ns, w in ((0, 512), (512, 128)):
                    for pc in range(3):
                        nc.tensor.matmul(
                            psv[:64, ns:ns + w],
                            xT_8[:, pc, :, vb * 64:(vb + 1) * 64],
                            w_v_8[:, pc, :, ns:ns + w],
                            start=(pc == 0), stop=(pc == 2),
                            perf_mode=DR,
                        )
                v_sb = work.tile([64, D], FP8, tag="probsT", bufs=4,
                                 name=f"v_sb{vb}")
                nc.vector.tensor_copy(out=v_sb[:], in_=psv[:64, :D])
                nc.sync.dma_start(
                    out=ag_in_vr[vb * 64:(vb + 1) * 64, :], in_=v_sb[:]
                )

            if SIM_NO_COLLECTIVE:
                nc.sync.dma_start(out=ag_out_v[0:FLATV], in_=ag_in_v[:])
                nc.sync.dma_start(out=ag_out_v[FLATV:2 * FLATV], in_=ag_in_v[:])
            else:
                nc.gpsimd.collective_compute(
                    "AllGather",
                    mybir.AluOpType.bypass,
                    replica_groups=REPLICA_GROUPS,
                    ins=[ag_in_v[:]],
                    outs=[ag_out_v[:]],
                )

            # last dim padded 81->82 so the kc-pair step (8*82=656) is %16==0
            # as DoubleRow's stationary AP requires.
            v_aug = const.tile([P, NK, H, 82], FP8, name="v_aug", tag="slot20")
            nc.gpsimd.memset(v_aug[:, :, :, DH:DH + 1], 1.0)
            for s in range(2):
                vsh = ag_out_v[s * FLATV:(s + 1) * FLATV].rearrange(
                    "(r f) -> r f", f=D
                )
                for rc in range(8):
                    nc.sync.dma_start(
                        out=v_aug[:, s * 8 + rc, :, 0:DH],
                        in_=vsh[rc * P:(rc + 1) * P, :].rearrange(
                            "p (h d) -> p h d", d=DH
                        ),
                    )

            # AU operands (deferred loads)
            w_ak_bf = const.tile([P, KC_C, D], BF16, name="w_ak_bf")
            nc.sync.dma_start(
                out=w_ak_bf[:], in_=wakb[:].rearrange("(c p) n -> p c n", p=P)
            )
            w_av_bf = const.tile([P, KC_C, D], BF16, name="w_av_bf")
            nc.sync.dma_start(
                out=w_av_bf[:], in_=wavb[:].rearrange("(c p) n -> p c n", p=P)
            )
            auT_bf = const.tile([P, KC_C, A], BF16, name="auT_bf")
            nc.sync.dma_start(
                out=auT_bf[:], in_=auTb[:].rearrange("(c p) n -> p c n", p=P)
            )
            t_sb = const.tile([P, 1], F32, name="t_sb")
            nc.sync.dma_start(out=t_sb[:], in_=temp[:].to_broadcast((P, 1)))
            alpha_s = const.tile([P, 1], F32, name="alpha_s")
            nc.vector.tensor_scalar_mul(alpha_s[:], t_sb[:], SCALE)

            # ------------- AU cross-attention constants -------------
            au_kT_s = const.tile([P, H, A], BF16, name="au_kT_s")
            nc.gpsimd.memset(au_kT_s[64:128, :, :], 0.0)
            for h in range(H):
                psak = ps_g.tile([P, R], F32, tag="psg", name=f"psak{h}")
                for kc in range(KC_C):
                    nc.tensor.matmul(
                        psak[:DH, 0:A],
                        w_ak_bf[:, kc, h * DH:(h + 1) * DH],
                        auT_bf[:, kc, :],
                        start=(kc == 0), stop=(kc == KC_C - 1),
                    )
                nc.vector.tensor_scalar_mul(
                    au_kT_s[:DH, h, :], psak[:DH, 0:A], alpha_s[:DH]
                )

            au_v_aug = const.tile([P, H, DH + 1], BF16, name="au_v_aug")
            nc.gpsimd.memset(au_v_aug[:], 0.0)
            nc.gpsimd.memset(au_v_aug[:A, :, DH:DH + 1], 1.0)
            psav = ps_g.tile([P, R], F32, tag="psg", name="psav")
            for ns, w in ((0, 512), (512, 128)):
                for kc in range(KC_C):
                    nc.tensor.matmul(
                        psav[:A, ns:ns + w],
                        auT_bf[:, kc, :],
                        w_av_bf[:, kc, ns:ns + w],
                        start=(kc == 0), stop=(kc == KC_C - 1),
                    )
            nc.vector.tensor_copy(
                out=au_v_aug[:A, :, 0:DH],
                in_=psav[:A, 0:D].rearrange("p (h d) -> p h d", d=DH),
            )

            # persistent AU probs tile: zeroed once, exps rewrite rows 0:16
            au_pT = const.tile([P, R], BF16, name="au_pT")
            nc.gpsimd.memset(au_pT[:, :], 0.0)

            # ------------- main self-attention -------------
            dram_hs_sums = dram.tile([H, R], BF16, name="dram_hs_sums")
            hs_keep = []

            def attn_head(h, fillers=()):
                fillers = list(fillers)
                pshs = ps_acc.tile([P, R], F32, tag="acc", name=f"pshs{h}")
                for c in range(NK // 2):
                    if fillers and c > 0:
                        budget = 1100
                        while fillers and budget > 0:
                            cost, fn = fillers.pop(0)
                            fn()
                            budget -= cost
                    pt = work.tile([P, 2, R], FP8, tag="probsT", bufs=4,
                                   name=f"pt{h}_{c}")
                    for j in range(2):
                        pslog = ps_a.tile([P, R], F32, tag="ps",
                                          name=f"pslog{h}_{c}_{j}")
                        kc = 2 * c + j
                        with tc.high_priority():
                            for qn in range(2):
                                nc.tensor.matmul(
                                    pslog[:, qn * 512:(qn + 1) * 512],
                                    kT_bf[:, h, kc * P:(kc + 1) * P],
                                    qT_bf[:, h, qn * 512:(qn + 1) * 512],
                                    start=True, stop=True,
                                )
                        nc.scalar.activation(out=pt[:, j, :], in_=pslog[:],
                                             func=AF.Exp, scale=SCALE,
                                             bias=exp_b_sb[:, 0:1])
                    with tc.high_priority():
                        for qn in range(2):
                            nc.tensor.matmul(
                                pshs[:DH + 1, qn * 512:(qn + 1) * 512],
                                v_aug[:, 2 * c:2 * c + 2, h, 0:DH + 1],
                                pt[:, :, qn * 512:(qn + 1) * 512],
                                start=(c == 0),
                                stop=(c == NK // 2 - 1),
                                perf_mode=DR,
                            )
                for cost, fn in fillers:
                    fn()
                hs_st = work.tile([P, R], BF16, tag="hs_keep", bufs=8,
                                  name=f"hs_st{h}")
                nc.vector.tensor_copy(out=hs_st[:DH + 1, :], in_=pshs[:DH + 1, :])
                if h < 6:
                    nc.sync.dma_start(out=dram_hs_sums[h], in_=hs_st[DH:DH + 1, :])
                hs_keep.append(hs_st)

            dram_rec_row = dram.tile([H, R], BF16, name="dram_rec_row")

            def row_recip_mul(sums_row, dst, src, h, name):
                """dst = src * (1/sums_row); recip on the [1,R] sums row, then
                a 2KB DRAM bounce for the partition broadcast. The sums row
                lives on partition 80, so it reaches partition 0 by DMA
                (engines cannot shift partitions)."""
                r16 = work.tile([1, R], BF16, tag="rr16", bufs=1,
                                name=f"{name}_16")
                nc.sync.dma_start(out=r16[:], in_=sums_row)
                rb = work.tile([1, R], BF16, tag="rrb", bufs=1, name=f"{name}_b")
                for qn in range(2):
                    sl = np.s_[:, qn * 512:(qn + 1) * 512]
                    rf = work.tile([1, 512], F32, tag="rrf", bufs=1,
                                   name=f"{name}_f{qn}")
                    nc.vector.tensor_copy(out=rf[:], in_=r16[sl])
                    ro = work.tile([1, 512], F32, tag="rro", bufs=1,
                                   name=f"{name}_o{qn}")
                    nc.vector.reciprocal_approx_fast(ro[:], rf[:])
                    nc.vector.tensor_copy(out=rb[sl], in_=ro[:])
                nc.sync.dma_start(out=dram_rec_row[h, :], in_=rb[0:1, :])
                bc = work.tile([DH, R], BF16, tag="bc", bufs=1, name=f"{name}_bc")
                nc.sync.dma_start(
                    out=bc[:], in_=dram_rec_row[h:h + 1, :].to_broadcast((DH, R))
                )
                nc.vector.tensor_mul(dst, src, bc[:])

            dram_au = dram.tile([H, DH + 1, R], BF16, name="dram_au")

            def au_head(h):
                psal = ps_g.tile([P, R], F32, tag="psg", name=f"psal{h}")
                for qn in range(2):
                    nc.tensor.matmul(
                        psal[:A, qn * 512:(qn + 1) * 512],
                        au_kT_s[:, h, :],
                        qT_bf[:, h, qn * 512:(qn + 1) * 512],
                        start=True, stop=True,
                    )
                nc.scalar.activation(out=au_pT[:A, :], in_=psal[:A, :],
                                     func=AF.Exp)
                psau = ps_g.tile([P, R], F32, tag="psg", name=f"psau{h}")
                for qn in range(2):
                    nc.tensor.matmul(
                        psau[:DH + 1, qn * 512:(qn + 1) * 512],
                        au_v_aug[:, h, :],
                        au_pT[:, qn * 512:(qn + 1) * 512],
                        start=True, stop=True,
                    )
                au_st = work.tile([P, R], BF16, tag="evac", bufs=1,
                                  name=f"au_st{h}")
                nc.vector.tensor_copy(out=au_st[:DH + 1, :], in_=psau[:DH + 1, :])
                nc.sync.dma_start(out=dram_au[h], in_=au_st[:DH + 1, :])

            # reciprocal chain: per-(head,query) sums -> 1/sum (bf16) in DRAM
            def recip_chain(sums_src, name, dma_eng):
                rc_in = work.tile([P, 64], BF16, tag="rc", bufs=1, name=f"{name}_in")
                for h in range(H):
                    dma_eng.dma_start(
                        out=rc_in[h * 16:(h + 1) * 16, :],
                        in_=sums_src(h),
                    )
                rc_f = work.tile([P, 64], F32, tag="rcf", bufs=1, name=f"{name}_f")
                nc.vector.tensor_copy(out=rc_f[:], in_=rc_in[:])
                rc_s = work.tile([P, 64], F32, tag="rcs", bufs=1, name=f"{name}_s")
                rc_o = work.tile([P, 64], F32, tag="rco", bufs=1, name=f"{name}_o")
                nc.vector.reciprocal_approx_accurate(rc_o[:], rc_f[:], rc_s[:])
                rc_b = work.tile([P, 64], BF16, tag="rcb", bufs=1, name=f"{name}_b")
                nc.vector.tensor_copy(out=rc_b[:], in_=rc_o[:])
                drec = dram.tile([H, R], BF16, name=f"{name}_dr")
                dma_eng.dma_start(
                    out=drec[:].rearrange("h (a j) -> (h a) j", j=64), in_=rc_b[:]
                )
                return drec

            au_rec_ref = []

            def au_finish():
                dram_au_rec = recip_chain(
                    lambda h: dram_au[h, DH, :].rearrange("(a j) -> a j", j=64),
                    "aurec", nc.sync,
                )
                nc.gpsimd.memset(au_hsT[64:128, :, :], 0.0)
                for h in range(H):
                    bc = work.tile([DH, R], BF16, tag="bc", bufs=1, name=f"aubc{h}")
                    nc.sync.dma_start(
                        out=bc[:], in_=dram_au_rec[h:h + 1, :].to_broadcast((DH, R))
                    )
                    au_ld = work.tile([DH, R], BF16, tag="evac", bufs=1,
                                      name=f"auld{h}")
                    nc.sync.dma_start(out=au_ld[:], in_=dram_au[h, 0:DH, :])
                    nc.vector.tensor_mul(au_hsT[:DH, h, :], au_ld[:], bc[:])
                au_rec_ref.append(dram_au_rec)

            au_hsT = const.tile([P, H, R], BF16, name="au_hsT", tag="slot16a")

            # ---- attention ladder: projections/AU heads pumped as <=1.1us
            # filler quanta inside each head's chunk loop so the exp stream
            # never drains the pslog double-buffer. ----
            attn_head(0, [
                (1070, lambda: k_head_qn(4, 0)), (1070, lambda: k_head_qn(4, 1)),
                (1070, lambda: k_head_qn(5, 0)), (1070, lambda: k_head_qn(5, 1)),
                (1070, lambda: k_head_qn(6, 0)), (1070, lambda: k_head_qn(6, 1)),
                (1070, lambda: k_head_qn(7, 0)), (1070, lambda: k_head_qn(7, 1)),
                (600, lambda: (k_send(ag_in_k2, ag_out_k2, 4),
                               kt_fill(4, ag_out_k2))),
            ])
            attn_head(1, [
                (1070, lambda: q_head_qn(1, 0)), (1070, lambda: q_head_qn(1, 1)),
                (900, lambda: au_head(0)), (900, lambda: au_head(1)),
                (1070, lambda: q_head_qn(2, 0)), (1070, lambda: q_head_qn(2, 1)),
            ])
            attn_head(2, [
                (1070, lambda: q_head_qn(3, 0)), (1070, lambda: q_head_qn(3, 1)),
                (900, lambda: au_head(2)), (900, lambda: au_head(3)),
                (1070, lambda: q_head_qn(4, 0)), (1070, lambda: q_head_qn(4, 1)),
            ])
            attn_head(3, [
                (1070, lambda: q_head_qn(5, 0)), (1070, lambda: q_head_qn(5, 1)),
                (900, lambda: au_head(4)), (900, lambda: au_head(5)),
                (1070, lambda: q_head_qn(6, 0)), (1070, lambda: q_head_qn(6, 1)),
            ])
            attn_head(4, [
                (1070, lambda: q_head_qn(7, 0)), (1070, lambda: q_head_qn(7, 1)),
                (900, lambda: au_head(6)), (900, lambda: au_head(7)),
            ])
            au_finish()

            # late loads: gate + out-proj weights (DMA slack mid-window)
            w_g1_hm = const.tile([P, H, G], BF16, name="w_g1_hm")
            nc.gpsimd.memset(w_g1_hm[64:128, :, :], 0.0)
            nc.sync.dma_start(out=w_g1_hm[:DH, :, :], in_=wg1hmb[:])
            w_g2_bf = const.tile([P, 3, D], BF16, name="w_g2_bf")
            nc.sync.dma_start(
                out=w_g2_bf[:, 0:2, :],
                in_=wg2b[0:256, :].rearrange("(c p) n -> p c n", p=P),
            )
            nc.sync.dma_start(out=w_g2_bf[:64, 2, :], in_=wg2b[256:320, :])
            nc.gpsimd.memset(w_g2_bf[64:128, 2, :], 0.0)
            w_out_hm = const.tile([P, H, D], BF16, name="w_out_hm")
            nc.gpsimd.memset(w_out_hm[64:128, :, :], 0.0)
            nc.sync.dma_start(out=w_out_hm[:DH, :, :], in_=wouthmb[:])
            b_g1_sb = const.tile([P, 3], F32, name="b_g1_sb")
            nc.vector.memset(b_g1_sb[:], 0.0)
            nc.sync.dma_start(
                out=b_g1_sb[:, 0:2], in_=b_g1[0:256].rearrange("(c p) -> p c", p=P)
            )
            nc.sync.dma_start(out=b_g1_sb[:64, 2:3], in_=b_g1[256:320][:, None])
            b_g2_hm = const.tile([P, H], F32, name="b_g2_hm")  # holds b_g2/2
            nc.vector.memset(b_g2_hm[:], 0.0)
            nc.sync.dma_start(
                out=b_g2_hm[:DH, :], in_=b_g2h[:].rearrange("(h p) -> p h", p=DH)
            )
            b_out_sb = const.tile([P, KC_D], F32, name="b_out_sb")
            nc.sync.dma_start(
                out=b_out_sb[:], in_=b_out[:].rearrange("(c p) -> p c", p=P)
            )

            # residual+bias rows preloaded so out-proj finishes don't wait DMA
            rx_t = {}
            for mo in range(KC_D):
                rx = work.tile([P, R], F32, tag="rx", bufs=3, name=f"rx{mo}")
                nc.sync.dma_start(out=rx[:], in_=xT[mo * P:(mo + 1) * P, :])
                nc.vector.tensor_scalar_add(rx[:], rx[:], b_out_sb[:, mo:mo + 1])
                rx_t[mo] = rx

            # ---- gate MLP (emitted here; runs under attn3-5 exp windows) ----
            siluT = const.tile([P, 3, R], BF16, name="siluT", tag="slot16q_silu")
            nc.gpsimd.memset(siluT[64:128, 2, :], 0.0)

            def l1_q(mo, qn):
                rows = 128 if mo < 2 else 64
                psl1 = ps_g.tile([P, R], F32, tag="psg", name=f"psl1{mo}_{qn}")
                for h in range(H):
                    nc.tensor.matmul(
                        psl1[:rows, 0:512],
                        w_g1_hm[:, h, mo * P:mo * P + rows],
                        au_hsT[:, h, qn * 512:(qn + 1) * 512],
                        start=(h == 0), stop=(h == H - 1),
                    )
                nc.scalar.activation(
                    out=siluT[:rows, mo, qn * 512:(qn + 1) * 512],
                    in_=psl1[:rows, 0:512],
                    func=AF.Silu, scale=0.125, bias=b_g1_sb[:rows, mo:mo + 1],
                )

            fusedA = const.tile([P, 6, R], BF16, name="fusedA", tag="slotx")
            nc.gpsimd.memset(fusedA[64:128, :, :], 0.0)
            fusedB6 = const.tile([P, R], BF16, name="fusedB6")
            nc.gpsimd.memset(fusedB6[64:128, :], 0.0)
            fusedB7 = const.tile([P, R], BF16, name="fusedB7")
            nc.gpsimd.memset(fusedB7[64:128, :], 0.0)

            def fused_sl(h):
                if h < 6:
                    return fusedA[:, h, :]
                return fusedB6[:, :] if h == 6 else fusedB7[:, :]

            def gate_head(h):
                # tanh(psg/2 + b_g2/2); gate*au_hs = (1+t)*au_hs/2
                psg = ps_g.tile([P, R], F32, tag="psg", name=f"psgate{h}")
                for qn in range(2):
                    for kc in range(3):
                        nc.tensor.matmul(
                            psg[:DH, qn * 512:(qn + 1) * 512],
                            w_g2_bf[:, kc, h * DH:(h + 1) * DH],
                            siluT[:, kc, qn * 512:(qn + 1) * 512],
                            start=(kc == 0), stop=(kc == 2),
                        )
                gateT = work.tile([DH, R], BF16, tag="gateT", bufs=1,
                                  name=f"gateT{h}")
                nc.scalar.activation(
                    out=gateT[:], in_=psg[:DH, :],
                    func=AF.Tanh, scale=0.5, bias=b_g2_hm[:DH, h:h + 1],
                )
                # fused = (gateT + 1) * au_hsT  (au_hsT carries x8 => x16 net)
                nc.vector.scalar_tensor_tensor(
                    out=fused_sl(h)[:DH, :], in0=gateT[:], scalar=1.0,
                    in1=au_hsT[:DH, h, :], op0=ALU.add, op1=ALU.mult,
                )

            attn_head(5, [
                (1700, lambda mo=mo, qn=qn: l1_q(mo, qn))
                for mo in range(3) for qn in range(2)
            ])
            attn_head(6, [
                (1300, lambda h=h: gate_head(h)) for h in range(H)
            ])

            # ---- hs recip chain A: heads 0..5; fused(h) += hs*rec ----
            rcA_in = work.tile([P, 64], BF16, tag="rc", bufs=1, name="rcA_in")
            for h in range(6):
                nc.sync.dma_start(
                    out=rcA_in[h * 16:(h + 1) * 16, :],
                    in_=dram_hs_sums[h, :].rearrange("(a j) -> a j", j=64),
                )
            rcA_f = work.tile([P, 64], F32, tag="rcf", bufs=1, name="rcA_f")
            nc.vector.memset(rcA_f[96:, :], 1.0)
            nc.vector.tensor_copy(out=rcA_f[:96, :], in_=rcA_in[:96, :])
            rcA_s = work.tile([P, 64], F32, tag="rcs", bufs=1, name="rcA_s")
            rcA_o = work.tile([P, 64], F32, tag="rco", bufs=1, name="rcA_o")
            nc.vector.reciprocal_approx_accurate(rcA_o[:], rcA_f[:], rcA_s[:])
            rcA_b = work.tile([P, 64], BF16, tag="rcb", bufs=1, name="rcA_b")
            nc.vector.tensor_copy(out=rcA_b[:96, :], in_=rcA_o[:96, :])
            dram_hs_rec = dram.tile([H, R], BF16, name="hsrec_dr")
            nc.sync.dma_start(
                out=dram_hs_rec[0:6, :].rearrange("h (a j) -> (h a) j", j=64),
                in_=rcA_b[:96, :],
            )
            for h in range(6):
                bch = work.tile([DH, R], BF16, tag="bc", bufs=1, name=f"hsbc{h}")
                nc.sync.dma_start(
                    out=bch[:], in_=dram_hs_rec[h:h + 1, :].to_broadcast((DH, R))
                )
                hs_st = hs_keep[h]
                nc.vector.tensor_mul(hs_st[:DH, :], hs_st[:DH, :], bch[:])
                nc.vector.tensor_add(
                    fused_sl(h)[:DH, :], fused_sl(h)[:DH, :], hs_st[:DH, :]
                )

            # ---- per-head tail recip for h6/h7 ----
            def recip_tail(h):
                # fast SBUF-only recip: sums row is hs_st row 80
                row_recip_mul(
                    hs_keep[h][DH:DH + 1, :], hs_keep[h][:DH, :],
                    hs_keep[h][:DH, :], h, f"rt{h}",
                )
                nc.vector.tensor_add(
                    fused_sl(h)[:DH, :], fused_sl(h)[:DH, :], hs_keep[h][:DH, :]
                )

            recip_tail(6)

            # ------------- output projection + residual -------------
            pso_t = {}

            PSO_POOL = {0: (ps_g, "psg"), 1: (ps_a, "ps"), 2: (ps_acc, "acc"),
                        3: (ps_a, "ps"), 4: (ps_g, "psg")}

            def out_partial(mo, h0, h1):
                if mo not in pso_t:
                    pool, tg = PSO_POOL[mo]
                    pso_t[mo] = pool.tile([P, R], F32, tag=tg, name=f"pso{mo}")
                t = pso_t[mo]
                for qn in range(2):
                    for h in range(h0, h1):
                        nc.tensor.matmul(
                            t[:, qn * 512:(qn + 1) * 512],
                            w_out_hm[:, h, mo * P:(mo + 1) * P],
                            fused_sl(h)[:, qn * 512:(qn + 1) * 512],
                            start=(h == 0), stop=(h == H - 1),
                        )

            def out_finish(mo):
                # out = pso/16 + (residual + b_out), in place on the rx tile
                rx = rx_t[mo]
                nc.vector.scalar_tensor_tensor(
                    out=rx[:], in0=pso_t[mo][:], scalar=0.0625,
                    in1=rx[:], op0=ALU.mult, op1=ALU.add,
                )
                nc.sync.dma_start(out=outT[mo * P:(mo + 1) * P, :], in_=rx[:])

            attn_head(7)
            # h0-6 partials for mo 0,1 run under attn7's exp window
            out_partial(0, 0, 7)
            out_partial(1, 0, 7)

            recip_tail(7)

            out_partial(0, 7, H)
            out_finish(0)
            for mo in range(2, KC_D):
                out_partial(mo, 0, 7)
                out_partial(mo - 1, 7, H)
                out_finish(mo - 1)
            out_partial(KC_D - 1, 7, H)
            out_finish(KC_D - 1)

    nc.finalize()
    return nc


_NC_CACHE = []


def get_program():
    if not _NC_CACHE:
        _NC_CACHE.append(_build_program())
    return _NC_CACHE[0]


def _bf(x):
    import ml_dtypes
    return np.ascontiguousarray(x.astype(ml_dtypes.bfloat16))


def _f8(x):
    import ml_dtypes
    return np.ascontiguousarray(x.astype(ml_dtypes.float8_e4m3))


def _pair_chunks(w, rows, cols, scale):
    """[rows<=768, cols] -> fp8 [128, 3, 2, cols] over d-chunk pairs."""
    out = np.zeros((P, 3, 2, cols), np.float32)
    for pc in range(3):
        for i in range(2):
            r0 = (2 * pc + i) * P
            r1 = min(r0 + P, rows)
            if r0 < rows:
                out[: r1 - r0, pc, i, :] = w[r0:r1, :]
    return _f8(out * scale)


def kernel(**inputs):
    f = lambda k: np.ascontiguousarray(np.asarray(inputs[k], dtype=np.float32))
    hidden = f("hidden_states")          # [4, 2048, 640]
    au = f("au_embedding")               # [4, 16, 768]
    w_g1 = f("w_g1")                     # [640, 320]
    w_out_w = f("w_out")                 # [640, 640]
    shared = {
        "wqb": _bf(f("w_q")),
        "wkb": _bf(f("w_k")),
        "wvb8": _pair_chunks(f("w_v"), D, D, 16.0),
        "wakb": _bf(f("w_ak")),
        "wavb": _bf(f("w_av") * 8.0),
        "wg1hmb": _bf(w_g1.reshape(H, DH, G).transpose(1, 0, 2)),
        "wg2b": _bf(f("w_g2")),
        "wouthmb": _bf(w_out_w.reshape(H, DH, D).transpose(1, 0, 2)),
        "b_g1": f("b_g1"),
        "b_g2h": f("b_g2") * 0.5,
        "b_out": f("b_out"),
        "temperature": f("temperature"),
    }
    in_maps = []
    for c in range(N_CORES):
        b, half = divmod(c, 2)
        m = dict(shared)
        xt = np.ascontiguousarray(hidden[b, half * R:(half + 1) * R, :].T)
        m["xT"] = xt
        m["xTb"] = _bf(xt)
        m["xTb8"] = _pair_chunks(xt, D, R, 1.0)
        m["auTb"] = _bf(np.ascontiguousarray(au[b].T))
        in_maps.append(m)

    nc = get_program()
    try:
        res = run_bass_kernel_spmd(nc, in_maps, core_ids=list(range(N_CORES)))
    except Exception:
        # transient device wedge (NRT_EXEC_UNIT_UNRECOVERABLE) — retry once
        import time as _time
        _time.sleep(10)
        res = run_bass_kernel_spmd(nc, in_maps, core_ids=list(range(N_CORES)))

    out = np.empty((B, S, D), dtype=np.float32)
    for c in range(N_CORES):
        b, half = divmod(c, 2)
        out[b, half * R:(half + 1) * R, :] = res.results[c]["outT"].T
    return out


# revision 47
# speedup vs baseline: 1.0361x; 1.0054x over previous
"""Trainium2 Bass kernel for nn_AUAttnProcessor (self-attn + AU cross-attn + gated fusion).

Sharding: 8 cores = 4 batches x 2 sequence-halves. Each core computes its
1024 query rows end-to-end in a feature-major ("transposed", [D, tokens])
dataflow. k/v are computed locally per half and AllGathered within the
2-core batch pair.

Numerics (validated vs reference on CPU, rel ~2.6e-3):
- bf16: x, w_q/w_k, q/k, logits, whole AU attention (w_av host-scaled x8
  so au_hs matches hs scale), hs, fused, w_out, out projection.
- fp8e4m3: v path (x8 input copy, w_v host-scaled x16, v, v_aug, its
  AllGather payload), main-attention probs.
- Main softmax: exp(SCALE*logits - 2) -- the -2 keeps probs under fp8 max
  (z reaches ~6.7); the factor e^-2 cancels exactly through the ones-column
  normalizer. AU probs stay bf16 (temperature-5 logits reach e^26).
- PV runs fp8 DoubleRow: probs tiles hold kc pairs [128, 2, R] so one
  matmul contracts 256 keys at 0.5 cycles/col.
- gate: sigmoid(x) = 0.5*(1+tanh(x/2)) -- Tanh shares the Exp activation
  table, so the only table switches are around the 3 Silu instructions.
  fused = (1+tanh)*au_hs8 + hs16 via one scalar_tensor_tensor per head.
- out = fused(16x) @ w_out -> evac (psum * 1/16 + residual + b_out).

Schedule: window is ACT(exp)-paced (~157us of activation work). k proj ->
AGk -> q h0 -> attention starts on kT slot 0 while everything else (q h1-7,
v proj, AU, gate, out-proj partials) fills PE slack under the exp window.
"""

import numpy as np

import concourse.bacc as bacc
import concourse.bass as bass
import concourse.tile as tile
from concourse import mybir
from concourse.bass_utils import run_bass_kernel_spmd

F32 = mybir.dt.float32
BF16 = mybir.dt.bfloat16
FP8 = mybir.dt.float8e4
AF = mybir.ActivationFunctionType
ALU = mybir.AluOpType
DR = mybir.MatmulPerfMode.DoubleRow

P = 128
B, S, D, C, A = 4, 2048, 640, 768, 16
H, DH = 8, 80
R = 1024          # rows (tokens) per core
G = 320           # gate hidden
KC_D = 5          # 640 / 128
KC_C = 6          # 768 / 128
NK = 16           # key chunks of 128 over S
SCALE = 1.0 / float(np.sqrt(DH))
EXP_BIAS = -2.0   # exp(scale*z - 2); cancels via normalizer
FLATK = DH * H * R        # elements of one kT shard (bf16)
FLATV = D * R             # elements of one v shard (fp8)

N_CORES = 8
REPLICA_GROUPS = [[0, 1], [2, 3], [4, 5], [6, 7]]

DEBUG = False
SIM_NO_COLLECTIVE = False  # replace AllGather with local DMAs for TimelineSim


def _build_program():
    nc = bacc.Bacc(None, target_bir_lowering=False)

    xT = nc.dram_tensor("xT", [D, R], F32, kind="ExternalInput")        # residual
    xTb = nc.dram_tensor("xTb", [D, R], BF16, kind="ExternalInput")
    xTb8 = nc.dram_tensor("xTb8", [P, 3, 2, R], FP8, kind="ExternalInput")
    auTb = nc.dram_tensor("auTb", [C, A], BF16, kind="ExternalInput")
    wqb = nc.dram_tensor("wqb", [D, D], BF16, kind="ExternalInput")
    wkb = nc.dram_tensor("wkb", [D, D], BF16, kind="ExternalInput")
    wvb8 = nc.dram_tensor("wvb8", [P, 3, 2, D], FP8, kind="ExternalInput")
    wakb = nc.dram_tensor("wakb", [C, D], BF16, kind="ExternalInput")
    wavb = nc.dram_tensor("wavb", [C, D], BF16, kind="ExternalInput")  # x8
    wg1hmb = nc.dram_tensor("wg1hmb", [DH, H, G], BF16, kind="ExternalInput")
    wg2b = nc.dram_tensor("wg2b", [G, D], BF16, kind="ExternalInput")
    wouthmb = nc.dram_tensor("wouthmb", [DH, H, D], BF16, kind="ExternalInput")
    b_g1 = nc.dram_tensor("b_g1", [G], F32, kind="ExternalInput")
    b_g2h = nc.dram_tensor("b_g2h", [D], F32, kind="ExternalInput")  # b_g2/2
    b_out = nc.dram_tensor("b_out", [D], F32, kind="ExternalInput")
    temp = nc.dram_tensor("temperature", [1], F32, kind="ExternalInput")
    outT = nc.dram_tensor("outT", [D, R], F32, kind="ExternalOutput")

    with tile.TileContext(nc) as tc:
        with (
            tc.tile_pool(name="const", bufs=1) as const,
            tc.tile_pool(name="work", bufs=2) as work,
            tc.tile_pool(name="ps_a", bufs=2, space="PSUM") as ps_a,
            tc.tile_pool(name="ps_acc", bufs=1, space="PSUM") as ps_acc,
            tc.tile_pool(name="ps_g", bufs=1, space="PSUM") as ps_g,
            tc.tile_pool(name="dram", bufs=1, space="DRAM") as dram,
        ):
            # ------------- load operands (k path first) -------------
            w_k_bf = const.tile([P, KC_D, D], BF16, name="w_k_bf")
            nc.sync.dma_start(out=w_k_bf[:, 0, :], in_=wkb[0:P, :])
            xT_bf = const.tile([P, KC_D, R], BF16, name="xT_bf", tag="slotx")
            nc.sync.dma_start(out=xT_bf[:, 0, :], in_=xTb[0:P, :])
            nc.sync.dma_start(
                out=w_k_bf[:, 1:, :],
                in_=wkb[P:, :].rearrange("(c p) n -> p c n", p=P),
            )
            nc.sync.dma_start(
                out=xT_bf[:, 1:, :],
                in_=xTb[P:, :].rearrange("(c p) n -> p c n", p=P),
            )
            w_q_bf = const.tile([P, KC_D, D], BF16, name="w_q_bf")
            nc.sync.dma_start(
                out=w_q_bf[:], in_=wqb[:].rearrange("(c p) n -> p c n", p=P)
            )
            exp_b_sb = const.tile([P, 1], F32, name="exp_b_sb")
            nc.vector.memset(exp_b_sb[:], EXP_BIAS)

            # ------------- k local projection + AllGather (split halves) ----
            FK2 = FLATK // 2
            ag_in_k1 = dram.tile([FK2], BF16, name="ag_in_k1")
            ag_in_k2 = dram.tile([FK2], BF16, name="ag_in_k2")
            ag_out_k1 = dram.tile([FLATK], BF16, name="ag_out_k1")
            ag_out_k2 = dram.tile([FLATK], BF16, name="ag_out_k2")
            ag_in_v = dram.tile([FLATV], FP8, name="ag_in_v")
            ag_out_v = dram.tile([2 * FLATV], FP8, name="ag_out_v")
            ag_in_vr = ag_in_v[:].rearrange("(r f) -> r f", f=D)

            kTl_bf = const.tile([P, H, R], BF16, name="kTl_bf", tag="slot16a")

            def k_head(h):
                psk = ps_a.tile([P, R], F32, tag="ps", name=f"psk{h}")
                for qn in range(2):
                    for kc in range(KC_D):
                        nc.tensor.matmul(
                            psk[:DH, qn * 512:(qn + 1) * 512],
                            w_k_bf[:, kc, h * DH:(h + 1) * DH],
                            xT_bf[:, kc, qn * 512:(qn + 1) * 512],
                            start=(kc == 0), stop=(kc == KC_D - 1),
                        )
                nc.vector.tensor_copy(out=kTl_bf[:DH, h, :], in_=psk[:DH, :])

            def k_head_qn(h, qn):
                # intra-window variant: single qn-half on the ps_g pool
                psk = ps_g.tile([P, R], F32, tag="psg", name=f"pskq{h}_{qn}")
                for kc in range(KC_D):
                    nc.tensor.matmul(
                        psk[:DH, 0:512],
                        w_k_bf[:, kc, h * DH:(h + 1) * DH],
                        xT_bf[:, kc, qn * 512:(qn + 1) * 512],
                        start=(kc == 0), stop=(kc == KC_D - 1),
                    )
                nc.vector.tensor_copy(
                    out=kTl_bf[:DH, h, qn * 512:(qn + 1) * 512],
                    in_=psk[:DH, 0:512],
                )

            def k_send(ag_in, ag_out, h0):
                nc.sync.dma_start(
                    out=ag_in[:].rearrange("(h p k) -> p h k", p=DH, k=R),
                    in_=kTl_bf[:DH, h0:h0 + 4, :],
                )
                if SIM_NO_COLLECTIVE:
                    nc.sync.dma_start(out=ag_out[0:FK2], in_=ag_in[:])
                    nc.sync.dma_start(out=ag_out[FK2:FLATK], in_=ag_in[:])
                else:
                    nc.gpsimd.collective_compute(
                        "AllGather",
                        mybir.AluOpType.bypass,
                        replica_groups=REPLICA_GROUPS,
                        ins=[ag_in[:]],
                        outs=[ag_out[:]],
                    )

            for h in range(4):
                k_head(h)
            k_send(ag_in_k1, ag_out_k1, 0)

            # ------------- q projection head 0 (window opener) -------------
            qT_bf = const.tile([P, H, R], BF16, name="qT_bf", tag="slot16q")
            nc.gpsimd.memset(qT_bf[64:128, :, :], 0.0)

            def q_head(h):
                psq = ps_acc.tile([P, R], F32, tag="acc", name=f"psq{h}")
                for qn in range(2):
                    for kc in range(KC_D):
                        nc.tensor.matmul(
                            psq[:DH, qn * 512:(qn + 1) * 512],
                            w_q_bf[:, kc, h * DH:(h + 1) * DH],
                            xT_bf[:, kc, qn * 512:(qn + 1) * 512],
                            start=(kc == 0), stop=(kc == KC_D - 1),
                        )
                nc.vector.tensor_copy(out=qT_bf[:DH, h, :], in_=psq[:DH, :])

            def q_head_qn(h, qn):
                # intra-window variant: single qn-half on the ps_g pool
                psq = ps_g.tile([P, R], F32, tag="psg", name=f"psqq{h}_{qn}")
                for kc in range(KC_D):
                    nc.tensor.matmul(
                        psq[:DH, 0:512],
                        w_q_bf[:, kc, h * DH:(h + 1) * DH],
                        xT_bf[:, kc, qn * 512:(qn + 1) * 512],
                        start=(kc == 0), stop=(kc == KC_D - 1),
                    )
                nc.vector.tensor_copy(
                    out=qT_bf[:DH, h, qn * 512:(qn + 1) * 512],
                    in_=psq[:DH, 0:512],
                )

            q_head(0)

            # ---- full kT from AllGather: window-critical DMA chain ----
            kT_bf = const.tile([P, H, S], BF16, name="kT_bf")
            nc.gpsimd.memset(kT_bf[64:128, :, :], 0.0)

            def kt_fill(hg, ag_out):
                for s in range(2):
                    nc.sync.dma_start(
                        out=kT_bf[:DH, hg:hg + 4, s * R:(s + 1) * R],
                        in_=ag_out[s * FK2:(s + 1) * FK2].rearrange(
                            "(h p k) -> p h k", p=DH, k=R
                        ),
                    )

            kt_fill(0, ag_out_k1)
            # fp8 v-path operands (deferred so k-path DMAs go first)
            xT_8 = const.tile([P, 3, 2, R], FP8, name="xT_8")
            nc.sync.dma_start(out=xT_8[:], in_=xTb8[:])
            w_v_8 = const.tile([P, 3, 2, D], FP8, name="w_v_8")
            nc.sync.dma_start(out=w_v_8[:], in_=wvb8[:])

            # ------------- v local projection (fp8 DoubleRow) + AllGather ----
            for vb in range(16):  # DoubleRow needs dst base 0, M<=64
                psv = ps_a.tile([P, R], F32, tag="ps", name=f"psv{vb}")
                for ns, w in ((0, 512), (512, 128)):
                    for pc in range(3):
                        nc.tensor.matmul(
                            psv[:64, ns:ns + w],
                            xT_8[:, pc, :, vb * 64:(vb + 1) * 64],
                            w_v_8[:, pc, :, ns:ns + w],
                            start=(pc == 0), stop=(pc == 2),
                            perf_mode=DR,
                        )
                v_sb = work.tile([64, D], FP8, tag="probsT", bufs=4,
                                 name=f"v_sb{vb}")
                nc.vector.tensor_copy(out=v_sb[:], in_=psv[:64, :D])
                nc.sync.dma_start(
                    out=ag_in_vr[vb * 64:(vb + 1) * 64, :], in_=v_sb[:]
                )

            if SIM_NO_COLLECTIVE:
                nc.sync.dma_start(out=ag_out_v[0:FLATV], in_=ag_in_v[:])
                nc.sync.dma_start(out=ag_out_v[FLATV:2 * FLATV], in_=ag_in_v[:])
            else:
                nc.gpsimd.collective_compute(
                    "AllGather",
                    mybir.AluOpType.bypass,
                    replica_groups=REPLICA_GROUPS,
                    ins=[ag_in_v[:]],
                    outs=[ag_out_v[:]],
                )

            # last dim padded 81->82 so the kc-pair step (8*82=656) is %16==0
            # as DoubleRow's stationary AP requires.
            v_aug = const.tile([P, NK, H, 82], FP8, name="v_aug", tag="slot20")
            nc.gpsimd.memset(v_aug[:, :, :, DH:DH + 1], 1.0)
            for s in range(2):
                vsh = ag_out_v[s * FLATV:(s + 1) * FLATV].rearrange(
                    "(r f) -> r f", f=D
                )
                for rc in range(8):
                    nc.sync.dma_start(
                        out=v_aug[:, s * 8 + rc, :, 0:DH],
                        in_=vsh[rc * P:(rc + 1) * P, :].rearrange(
                            "p (h d) -> p h d", d=DH
                        ),
                    )

            # AU operands (deferred loads)
            w_ak_bf = const.tile([P, KC_C, D], BF16, name="w_ak_bf")
            nc.sync.dma_start(
                out=w_ak_bf[:], in_=wakb[:].rearrange("(c p) n -> p c n", p=P)
            )
            w_av_bf = const.tile([P, KC_C, D], BF16, name="w_av_bf")
            nc.sync.dma_start(
                out=w_av_bf[:], in_=wavb[:].rearrange("(c p) n -> p c n", p=P)
            )
            auT_bf = const.tile([P, KC_C, A], BF16, name="auT_bf")
            nc.sync.dma_start(
                out=auT_bf[:], in_=auTb[:].rearrange("(c p) n -> p c n", p=P)
            )
            t_sb = const.tile([P, 1], F32, name="t_sb")
            nc.sync.dma_start(out=t_sb[:], in_=temp[:].to_broadcast((P, 1)))
            alpha_s = const.tile([P, 1], F32, name="alpha_s")
            nc.vector.tensor_scalar_mul(alpha_s[:], t_sb[:], SCALE)

            # ------------- AU cross-attention constants -------------
            au_kT_s = const.tile([P, H, A], BF16, name="au_kT_s")
            nc.gpsimd.memset(au_kT_s[64:128, :, :], 0.0)
            for h in range(H):
                psak = ps_g.tile([P, R], F32, tag="psg", name=f"psak{h}")
                for kc in range(KC_C):
                    nc.tensor.matmul(
                        psak[:DH, 0:A],
                        w_ak_bf[:, kc, h * DH:(h + 1) * DH],
                        auT_bf[:, kc, :],
                        start=(kc == 0), stop=(kc == KC_C - 1),
                    )
                nc.vector.tensor_scalar_mul(
                    au_kT_s[:DH, h, :], psak[:DH, 0:A], alpha_s[:DH]
                )

            au_v_aug = const.tile([P, H, DH + 1], BF16, name="au_v_aug")
            nc.gpsimd.memset(au_v_aug[:], 0.0)
            nc.gpsimd.memset(au_v_aug[:A, :, DH:DH + 1], 1.0)
            psav = ps_g.tile([P, R], F32, tag="psg", name="psav")
            for ns, w in ((0, 512), (512, 128)):
                for kc in range(KC_C):
                    nc.tensor.matmul(
                        psav[:A, ns:ns + w],
                        auT_bf[:, kc, :],
                        w_av_bf[:, kc, ns:ns + w],
                        start=(kc == 0), stop=(kc == KC_C - 1),
                    )
            nc.vector.tensor_copy(
                out=au_v_aug[:A, :, 0:DH],
                in_=psav[:A, 0:D].rearrange("p (h d) -> p h d", d=DH),
            )

            # persistent AU probs tile: zeroed once, exps rewrite rows 0:16
            au_pT = const.tile([P, R], BF16, name="au_pT")
            nc.gpsimd.memset(au_pT[:, :], 0.0)

            # ------------- main self-attention -------------
            dram_hs_sums = dram.tile([H, R], BF16, name="dram_hs_sums")
            hs_keep = []

            def attn_head(h, fillers=()):
                fillers = list(fillers)
                pshs = ps_acc.tile([P, R], F32, tag="acc", name=f"pshs{h}")
                for c in range(NK // 2):
                    if fillers and c > 0:
                        budget = 1100
                        while fillers and budget > 0:
                            cost, fn = fillers.pop(0)
                            fn()
                            budget -= cost
                    pt = work.tile([P, 2, R], FP8, tag="probsT", bufs=4,
                                   name=f"pt{h}_{c}")
                    for j in range(2):
                        pslog = ps_a.tile([P, R], F32, tag="ps",
                                          name=f"pslog{h}_{c}_{j}")
                        kc = 2 * c + j
                        with tc.high_priority():
                            for qn in range(2):
                                nc.tensor.matmul(
                                    pslog[:, qn * 512:(qn + 1) * 512],
                                    kT_bf[:, h, kc * P:(kc + 1) * P],
                                    qT_bf[:, h, qn * 512:(qn + 1) * 512],
                                    start=True, stop=True,
                                )
                        nc.scalar.activation(out=pt[:, j, :], in_=pslog[:],
                                             func=AF.Exp, scale=SCALE,
                                             bias=exp_b_sb[:, 0:1])
                    with tc.high_priority():
                        for qn in range(2):
                            nc.tensor.matmul(
                                pshs[:DH + 1, qn * 512:(qn + 1) * 512],
                                v_aug[:, 2 * c:2 * c + 2, h, 0:DH + 1],
                                pt[:, :, qn * 512:(qn + 1) * 512],
                                start=(c == 0),
                                stop=(c == NK // 2 - 1),
                                perf_mode=DR,
                            )
                for cost, fn in fillers:
                    fn()
                hs_st = work.tile([P, R], BF16, tag="hs_keep", bufs=8,
                                  name=f"hs_st{h}")
                nc.vector.tensor_copy(out=hs_st[:DH + 1, :], in_=pshs[:DH + 1, :])
                if h < 6:
                    nc.sync.dma_start(out=dram_hs_sums[h], in_=hs_st[DH:DH + 1, :])
                hs_keep.append(hs_st)

            dram_rec_row = dram.tile([H, R], BF16, name="dram_rec_row")

            def row_recip_mul(sums_row, dst, src, h, name):
                """dst = src * (1/sums_row); recip on the [1,R] sums row, then
                a 2KB DRAM bounce for the partition broadcast. The sums row
                lives on partition 80, so it reaches partition 0 by DMA
                (engines cannot shift partitions)."""
                r16 = work.tile([1, R], BF16, tag="rr16", bufs=1,
                                name=f"{name}_16")
                nc.sync.dma_start(out=r16[:], in_=sums_row)
                rb = work.tile([1, R], BF16, tag="rrb", bufs=1, name=f"{name}_b")
                for qn in range(2):
                    sl = np.s_[:, qn * 512:(qn + 1) * 512]
                    rf = work.tile([1, 512], F32, tag="rrf", bufs=1,
                                   name=f"{name}_f{qn}")
                    nc.vector.tensor_copy(out=rf[:], in_=r16[sl])
                    ro = work.tile([1, 512], F32, tag="rro", bufs=1,
                                   name=f"{name}_o{qn}")
                    nc.vector.reciprocal_approx_fast(ro[:], rf[:])
                    nc.vector.tensor_copy(out=rb[sl], in_=ro[:])
                nc.sync.dma_start(out=dram_rec_row[h, :], in_=rb[0:1, :])
                bc = work.tile([DH, R], BF16, tag="bc", bufs=1, name=f"{name}_bc")
                nc.sync.dma_start(
                    out=bc[:], in_=dram_rec_row[h:h + 1, :].to_broadcast((DH, R))
                )
                nc.vector.tensor_mul(dst, src, bc[:])

            dram_au = dram.tile([H, DH + 1, R], BF16, name="dram_au")

            def au_head(h):
                psal = ps_g.tile([P, R], F32, tag="psg", name=f"psal{h}")
                for qn in range(2):
                    nc.tensor.matmul(
                        psal[:A, qn * 512:(qn + 1) * 512],
                        au_kT_s[:, h, :],
                        qT_bf[:, h, qn * 512:(qn + 1) * 512],
                        start=True, stop=True,
                    )
                nc.scalar.activation(out=au_pT[:A, :], in_=psal[:A, :],
                                     func=AF.Exp)
                psau = ps_g.tile([P, R], F32, tag="psg", name=f"psau{h}")
                for qn in range(2):
                    nc.tensor.matmul(
                        psau[:DH + 1, qn * 512:(qn + 1) * 512],
                        au_v_aug[:, h, :],
                        au_pT[:, qn * 512:(qn + 1) * 512],
                        start=True, stop=True,
                    )
                au_st = work.tile([P, R], BF16, tag="evac", bufs=1,
                                  name=f"au_st{h}")
                nc.vector.tensor_copy(out=au_st[:DH + 1, :], in_=psau[:DH + 1, :])
                nc.sync.dma_start(out=dram_au[h], in_=au_st[:DH + 1, :])

            # reciprocal chain: per-(head,query) sums -> 1/sum (bf16) in DRAM
            def recip_chain(sums_src, name, dma_eng):
                rc_in = work.tile([P, 64], BF16, tag="rc", bufs=1, name=f"{name}_in")
                for h in range(H):
                    dma_eng.dma_start(
                        out=rc_in[h * 16:(h + 1) * 16, :],
                        in_=sums_src(h),
                    )
                rc_f = work.tile([P, 64], F32, tag="rcf", bufs=1, name=f"{name}_f")
                nc.vector.tensor_copy(out=rc_f[:], in_=rc_in[:])
                rc_s = work.tile([P, 64], F32, tag="rcs", bufs=1, name=f"{name}_s")
                rc_o = work.tile([P, 64], F32, tag="rco", bufs=1, name=f"{name}_o")
                nc.vector.reciprocal_approx_accurate(rc_o[:], rc_f[:], rc_s[:])
                rc_b = work.tile([P, 64], BF16, tag="rcb", bufs=1, name=f"{name}_b")
                nc.vector.tensor_copy(out=rc_b[:], in_=rc_o[:])
                drec = dram.tile([H, R], BF16, name=f"{name}_dr")
                dma_eng.dma_start(
                    out=drec[:].rearrange("h (a j) -> (h a) j", j=64), in_=rc_b[:]
                )
                return drec

            au_rec_ref = []

            def au_finish():
                dram_au_rec = recip_chain(
                    lambda h: dram_au[h, DH, :].rearrange("(a j) -> a j", j=64),
                    "aurec", nc.sync,
                )
                nc.gpsimd.memset(au_hsT[64:128, :, :], 0.0)
                for h in range(H):
                    bc = work.tile([DH, R], BF16, tag="bc", bufs=1, name=f"aubc{h}")
                    nc.sync.dma_start(
                        out=bc[:], in_=dram_au_rec[h:h + 1, :].to_broadcast((DH, R))
                    )
                    au_ld = work.tile([DH, R], BF16, tag="evac", bufs=1,
                                      name=f"auld{h}")
                    nc.sync.dma_start(out=au_ld[:], in_=dram_au[h, 0:DH, :])
                    nc.vector.tensor_mul(au_hsT[:DH, h, :], au_ld[:], bc[:])
                au_rec_ref.append(dram_au_rec)

            au_hsT = const.tile([P, H, R], BF16, name="au_hsT", tag="slot16a")

            # ---- attention ladder: projections/AU heads pumped as <=1.1us
            # filler quanta inside each head's chunk loop so the exp stream
            # never drains the pslog double-buffer. ----
            attn_head(0, [
                (1070, lambda: k_head_qn(4, 0)), (1070, lambda: k_head_qn(4, 1)),
                (1070, lambda: k_head_qn(5, 0)), (1070, lambda: k_head_qn(5, 1)),
                (1070, lambda: k_head_qn(6, 0)), (1070, lambda: k_head_qn(6, 1)),
                (1070, lambda: k_head_qn(7, 0)), (1070, lambda: k_head_qn(7, 1)),
                (600, lambda: (k_send(ag_in_k2, ag_out_k2, 4),
                               kt_fill(4, ag_out_k2))),
                (1070, lambda: q_head_qn(1, 0)), (1070, lambda: q_head_qn(1, 1)),
            ])
            attn_head(1, [
                (900, lambda: au_head(0)), (900, lambda: au_head(1)),
                (1070, lambda: q_head_qn(2, 0)), (1070, lambda: q_head_qn(2, 1)),
            ])
            attn_head(2, [
                (1070, lambda: q_head_qn(3, 0)), (1070, lambda: q_head_qn(3, 1)),
                (900, lambda: au_head(2)), (900, lambda: au_head(3)),
                (1070, lambda: q_head_qn(4, 0)), (1070, lambda: q_head_qn(4, 1)),
            ])
            attn_head(3, [
                (1070, lambda: q_head_qn(5, 0)), (1070, lambda: q_head_qn(5, 1)),
                (900, lambda: au_head(4)), (900, lambda: au_head(5)),
                (1070, lambda: q_head_qn(6, 0)), (1070, lambda: q_head_qn(6, 1)),
            ])
            attn_head(4, [
                (1070, lambda: q_head_qn(7, 0)), (1070, lambda: q_head_qn(7, 1)),
                (900, lambda: au_head(6)), (900, lambda: au_head(7)),
            ])
            au_finish()

            # late loads: gate + out-proj weights (DMA slack mid-window)
            w_g1_hm = const.tile([P, H, G], BF16, name="w_g1_hm")
            nc.gpsimd.memset(w_g1_hm[64:128, :, :], 0.0)
            nc.sync.dma_start(out=w_g1_hm[:DH, :, :], in_=wg1hmb[:])
            w_g2_bf = const.tile([P, 3, D], BF16, name="w_g2_bf")
            nc.sync.dma_start(
                out=w_g2_bf[:, 0:2, :],
                in_=wg2b[0:256, :].rearrange("(c p) n -> p c n", p=P),
            )
            nc.sync.dma_start(out=w_g2_bf[:64, 2, :], in_=wg2b[256:320, :])
            nc.gpsimd.memset(w_g2_bf[64:128, 2, :], 0.0)
            w_out_hm = const.tile([P, H, D], BF16, name="w_out_hm")
            nc.gpsimd.memset(w_out_hm[64:128, :, :], 0.0)
            nc.sync.dma_start(out=w_out_hm[:DH, :, :], in_=wouthmb[:])
            b_g1_sb = const.tile([P, 3], F32, name="b_g1_sb")
            nc.vector.memset(b_g1_sb[:], 0.0)
            nc.sync.dma_start(
                out=b_g1_sb[:, 0:2], in_=b_g1[0:256].rearrange("(c p) -> p c", p=P)
            )
            nc.sync.dma_start(out=b_g1_sb[:64, 2:3], in_=b_g1[256:320][:, None])
            b_g2_hm = const.tile([P, H], F32, name="b_g2_hm")  # holds b_g2/2
            nc.vector.memset(b_g2_hm[:], 0.0)
            nc.sync.dma_start(
                out=b_g2_hm[:DH, :], in_=b_g2h[:].rearrange("(h p) -> p h", p=DH)
            )
            b_out_sb = const.tile([P, KC_D], F32, name="b_out_sb")
            nc.sync.dma_start(
                out=b_out_sb[:], in_=b_out[:].rearrange("(c p) -> p c", p=P)
            )

            # residual+bias rows preloaded so out-proj finishes don't wait DMA
            rx_t = {}
            for mo in range(KC_D):
                rx = work.tile([P, R], F32, tag="rx", bufs=3, name=f"rx{mo}")
                nc.sync.dma_start(out=rx[:], in_=xT[mo * P:(mo + 1) * P, :])
                nc.vector.tensor_scalar_add(rx[:], rx[:], b_out_sb[:, mo:mo + 1])
                rx_t[mo] = rx

            # ---- gate MLP (emitted here; runs under attn3-5 exp windows) ----
            siluT = const.tile([P, 3, R], BF16, name="siluT", tag="slot16q_silu")
            nc.gpsimd.memset(siluT[64:128, 2, :], 0.0)

            def l1_q(mo, qn):
                rows = 128 if mo < 2 else 64
                psl1 = ps_g.tile([P, R], F32, tag="psg", name=f"psl1{mo}_{qn}")
                for h in range(H):
                    nc.tensor.matmul(
                        psl1[:rows, 0:512],
                        w_g1_hm[:, h, mo * P:mo * P + rows],
                        au_hsT[:, h, qn * 512:(qn + 1) * 512],
                        start=(h == 0), stop=(h == H - 1),
                    )
                nc.scalar.activation(
                    out=siluT[:rows, mo, qn * 512:(qn + 1) * 512],
                    in_=psl1[:rows, 0:512],
                    func=AF.Silu, scale=0.125, bias=b_g1_sb[:rows, mo:mo + 1],
                )

            fusedA = const.tile([P, 6, R], BF16, name="fusedA", tag="slotx")
            nc.gpsimd.memset(fusedA[64:128, :, :], 0.0)
            fusedB6 = const.tile([P, R], BF16, name="fusedB6")
            nc.gpsimd.memset(fusedB6[64:128, :], 0.0)
            fusedB7 = const.tile([P, R], BF16, name="fusedB7")
            nc.gpsimd.memset(fusedB7[64:128, :], 0.0)

            def fused_sl(h):
                if h < 6:
                    return fusedA[:, h, :]
                return fusedB6[:, :] if h == 6 else fusedB7[:, :]

            def gate_head(h):
                # tanh(psg/2 + b_g2/2); gate*au_hs = (1+t)*au_hs/2
                psg = ps_g.tile([P, R], F32, tag="psg", name=f"psgate{h}")
                for qn in range(2):
                    for kc in range(3):
                        nc.tensor.matmul(
                            psg[:DH, qn * 512:(qn + 1) * 512],
                            w_g2_bf[:, kc, h * DH:(h + 1) * DH],
                            siluT[:, kc, qn * 512:(qn + 1) * 512],
                            start=(kc == 0), stop=(kc == 2),
                        )
                gateT = work.tile([DH, R], BF16, tag="gateT", bufs=1,
                                  name=f"gateT{h}")
                nc.scalar.activation(
                    out=gateT[:], in_=psg[:DH, :],
                    func=AF.Tanh, scale=0.5, bias=b_g2_hm[:DH, h:h + 1],
                )
                # fused = (gateT + 1) * au_hsT  (au_hsT carries x8 => x16 net)
                nc.vector.scalar_tensor_tensor(
                    out=fused_sl(h)[:DH, :], in0=gateT[:], scalar=1.0,
                    in1=au_hsT[:DH, h, :], op0=ALU.add, op1=ALU.mult,
                )

            attn_head(5, [
                (1700, lambda mo=mo, qn=qn: l1_q(mo, qn))
                for mo in range(3) for qn in range(2)
            ])
            attn_head(6, [
                (1300, lambda h=h: gate_head(h)) for h in range(H)
            ])

            # ---- hs recip chain A: heads 0..5; fused(h) += hs*rec ----
            rcA_in = work.tile([P, 64], BF16, tag="rc", bufs=1, name="rcA_in")
            for h in range(6):
                nc.sync.dma_start(
                    out=rcA_in[h * 16:(h + 1) * 16, :],
                    in_=dram_hs_sums[h, :].rearrange("(a j) -> a j", j=64),
                )
            rcA_f = work.tile([P, 64], F32, tag="rcf", bufs=1, name="rcA_f")
            nc.vector.memset(rcA_f[96:, :], 1.0)
            nc.vector.tensor_copy(out=rcA_f[:96, :], in_=rcA_in[:96, :])
            rcA_s = work.tile([P, 64], F32, tag="rcs", bufs=1, name="rcA_s")
            rcA_o = work.tile([P, 64], F32, tag="rco", bufs=1, name="rcA_o")
            nc.vector.reciprocal_approx_accurate(rcA_o[:], rcA_f[:], rcA_s[:])
            rcA_b = work.tile([P, 64], BF16, tag="rcb", bufs=1, name="rcA_b")
            nc.vector.tensor_copy(out=rcA_b[:96, :], in_=rcA_o[:96, :])
            dram_hs_rec = dram.tile([H, R], BF16, name="hsrec_dr")
            nc.sync.dma_start(
                out=dram_hs_rec[0:6, :].rearrange("h (a j) -> (h a) j", j=64),
                in_=rcA_b[:96, :],
            )
            for h in range(6):
                bch = work.tile([DH, R], BF16, tag="bc", bufs=1, name=f"hsbc{h}")
                nc.sync.dma_start(
                    out=bch[:], in_=dram_hs_rec[h:h + 1, :].to_broadcast((DH, R))
                )
                hs_st = hs_keep[h]
                nc.vector.tensor_mul(hs_st[:DH, :], hs_st[:DH, :], bch[:])
                nc.vector.tensor_add(
                    fused_sl(h)[:DH, :], fused_sl(h)[:DH, :], hs_st[:DH, :]
                )

            # ---- per-head tail recip for h6/h7 ----
            def recip_tail(h):
                # fast SBUF-only recip: sums row is hs_st row 80
                row_recip_mul(
                    hs_keep[h][DH:DH + 1, :], hs_keep[h][:DH, :],
                    hs_keep[h][:DH, :], h, f"rt{h}",
                )
                nc.vector.tensor_add(
                    fused_sl(h)[:DH, :], fused_sl(h)[:DH, :], hs_keep[h][:DH, :]
                )

            recip_tail(6)

            # ------------- output projection + residual -------------
            pso_t = {}

            PSO_POOL = {0: (ps_g, "psg"), 1: (ps_a, "ps"), 2: (ps_acc, "acc"),
                        3: (ps_a, "ps"), 4: (ps_g, "psg")}

            def out_partial(mo, h0, h1):
                if mo not in pso_t:
                    pool, tg = PSO_POOL[mo]
                    pso_t[mo] = pool.tile([P, R], F32, tag=tg, name=f"pso{mo}")
                t = pso_t[mo]
                for qn in range(2):
                    for h in range(h0, h1):
                        nc.tensor.matmul(
                            t[:, qn * 512:(qn + 1) * 512],
                            w_out_hm[:, h, mo * P:(mo + 1) * P],
                            fused_sl(h)[:, qn * 512:(qn + 1) * 512],
                            start=(h == 0), stop=(h == H - 1),
                        )

            def out_finish(mo):
                # out = pso/16 + (residual + b_out), in place on the rx tile
                rx = rx_t[mo]
                nc.vector.scalar_tensor_tensor(
                    out=rx[:], in0=pso_t[mo][:], scalar=0.0625,
                    in1=rx[:], op0=ALU.mult, op1=ALU.add,
                )
                nc.sync.dma_start(out=outT[mo * P:(mo + 1) * P, :], in_=rx[:])

            attn_head(7)
            # h0-6 partials for mo 0,1 run under attn7's exp window
            out_partial(0, 0, 7)
            out_partial(1, 0, 7)

            recip_tail(7)

            out_partial(0, 7, H)
            out_finish(0)
            for mo in range(2, KC_D):
                out_partial(mo, 0, 7)
                out_partial(mo - 1, 7, H)
                out_finish(mo - 1)
            out_partial(KC_D - 1, 7, H)
            out_finish(KC_D - 1)

    nc.finalize()
    return nc


_NC_CACHE = []


def get_program():
    if not _NC_CACHE:
        _NC_CACHE.append(_build_program())
    return _NC_CACHE[0]


def _bf(x):
    import ml_dtypes
    return np.ascontiguousarray(x.astype(ml_dtypes.bfloat16))


def _f8(x):
    import ml_dtypes
    return np.ascontiguousarray(x.astype(ml_dtypes.float8_e4m3))


def _pair_chunks(w, rows, cols, scale):
    """[rows<=768, cols] -> fp8 [128, 3, 2, cols] over d-chunk pairs."""
    out = np.zeros((P, 3, 2, cols), np.float32)
    for pc in range(3):
        for i in range(2):
            r0 = (2 * pc + i) * P
            r1 = min(r0 + P, rows)
            if r0 < rows:
                out[: r1 - r0, pc, i, :] = w[r0:r1, :]
    return _f8(out * scale)


def kernel(**inputs):
    f = lambda k: np.ascontiguousarray(np.asarray(inputs[k], dtype=np.float32))
    hidden = f("hidden_states")          # [4, 2048, 640]
    au = f("au_embedding")               # [4, 16, 768]
    w_g1 = f("w_g1")                     # [640, 320]
    w_out_w = f("w_out")                 # [640, 640]
    shared = {
        "wqb": _bf(f("w_q")),
        "wkb": _bf(f("w_k")),
        "wvb8": _pair_chunks(f("w_v"), D, D, 16.0),
        "wakb": _bf(f("w_ak")),
        "wavb": _bf(f("w_av") * 8.0),
        "wg1hmb": _bf(w_g1.reshape(H, DH, G).transpose(1, 0, 2)),
        "wg2b": _bf(f("w_g2")),
        "wouthmb": _bf(w_out_w.reshape(H, DH, D).transpose(1, 0, 2)),
        "b_g1": f("b_g1"),
        "b_g2h": f("b_g2") * 0.5,
        "b_out": f("b_out"),
        "temperature": f("temperature"),
    }
    in_maps = []
    for c in range(N_CORES):
        b, half = divmod(c, 2)
        m = dict(shared)
        xt = np.ascontiguousarray(hidden[b, half * R:(half + 1) * R, :].T)
        m["xT"] = xt
        m["xTb"] = _bf(xt)
        m["xTb8"] = _pair_chunks(xt, D, R, 1.0)
        m["auTb"] = _bf(np.ascontiguousarray(au[b].T))
        in_maps.append(m)

    nc = get_program()
    try:
        res = run_bass_kernel_spmd(nc, in_maps, core_ids=list(range(N_CORES)))
    except Exception:
        # transient device wedge (NRT_EXEC_UNIT_UNRECOVERABLE) — retry once
        import time as _time
        _time.sleep(10)
        res = run_bass_kernel_spmd(nc, in_maps, core_ids=list(range(N_CORES)))

    out = np.empty((B, S, D), dtype=np.float32)
    for c in range(N_CORES):
        b, half = divmod(c, 2)
        out[b, half * R:(half + 1) * R, :] = res.results[c]["outT"].T
    return out


# revision 48
# speedup vs baseline: 1.0480x; 1.0115x over previous
"""Trainium2 Bass kernel for nn_AUAttnProcessor (self-attn + AU cross-attn + gated fusion).

Sharding: 8 cores = 4 batches x 2 sequence-halves. Each core computes its
1024 query rows end-to-end in a feature-major ("transposed", [D, tokens])
dataflow. k/v are computed locally per half and AllGathered within the
2-core batch pair.

Numerics (validated vs reference on CPU, rel ~2.6e-3):
- bf16: x, w_q/w_k, q/k, logits, whole AU attention (w_av host-scaled x8
  so au_hs matches hs scale), hs, fused, w_out, out projection.
- fp8e4m3: v path (x8 input copy, w_v host-scaled x16, v, v_aug, its
  AllGather payload), main-attention probs.
- Main softmax: exp(SCALE*logits - 2) -- the -2 keeps probs under fp8 max
  (z reaches ~6.7); the factor e^-2 cancels exactly through the ones-column
  normalizer. AU probs stay bf16 (temperature-5 logits reach e^26).
- PV runs fp8 DoubleRow: probs tiles hold kc pairs [128, 2, R] so one
  matmul contracts 256 keys at 0.5 cycles/col.
- gate: sigmoid(x) = 0.5*(1+tanh(x/2)) -- Tanh shares the Exp activation
  table, so the only table switches are around the 3 Silu instructions.
  fused = (1+tanh)*au_hs8 + hs16 via one scalar_tensor_tensor per head.
- out = fused(16x) @ w_out -> evac (psum * 1/16 + residual + b_out).

Schedule: window is ACT(exp)-paced (~157us of activation work). k proj ->
AGk -> q h0 -> attention starts on kT slot 0 while everything else (q h1-7,
v proj, AU, gate, out-proj partials) fills PE slack under the exp window.
"""

import numpy as np

import concourse.bacc as bacc
import concourse.bass as bass
import concourse.tile as tile
from concourse import mybir
from concourse.bass_utils import run_bass_kernel_spmd

F32 = mybir.dt.float32
BF16 = mybir.dt.bfloat16
FP8 = mybir.dt.float8e4
AF = mybir.ActivationFunctionType
ALU = mybir.AluOpType
DR = mybir.MatmulPerfMode.DoubleRow

P = 128
B, S, D, C, A = 4, 2048, 640, 768, 16
H, DH = 8, 80
R = 1024          # rows (tokens) per core
G = 320           # gate hidden
KC_D = 5          # 640 / 128
KC_C = 6          # 768 / 128
NK = 16           # key chunks of 128 over S
SCALE = 1.0 / float(np.sqrt(DH))
EXP_BIAS = -2.0   # exp(scale*z - 2); cancels via normalizer
FLATK = DH * H * R        # elements of one kT shard (bf16)
FLATV = D * R             # elements of one v shard (fp8)

N_CORES = 8
REPLICA_GROUPS = [[0, 1], [2, 3], [4, 5], [6, 7]]

DEBUG = False
SIM_NO_COLLECTIVE = False  # replace AllGather with local DMAs for TimelineSim


def _build_program():
    nc = bacc.Bacc(None, target_bir_lowering=False)

    xT = nc.dram_tensor("xT", [D, R], F32, kind="ExternalInput")        # residual
    xTb = nc.dram_tensor("xTb", [D, R], BF16, kind="ExternalInput")
    xTb8 = nc.dram_tensor("xTb8", [P, 3, 2, R], FP8, kind="ExternalInput")
    auTb = nc.dram_tensor("auTb", [C, A], BF16, kind="ExternalInput")
    wqb = nc.dram_tensor("wqb", [D, D], BF16, kind="ExternalInput")
    wkb = nc.dram_tensor("wkb", [D, D], BF16, kind="ExternalInput")
    wvb8 = nc.dram_tensor("wvb8", [P, 3, 2, D], FP8, kind="ExternalInput")
    wakb = nc.dram_tensor("wakb", [C, D], BF16, kind="ExternalInput")
    wavb = nc.dram_tensor("wavb", [C, D], BF16, kind="ExternalInput")  # x8
    wg1hmb = nc.dram_tensor("wg1hmb", [DH, H, G], BF16, kind="ExternalInput")
    wg2b = nc.dram_tensor("wg2b", [G, D], BF16, kind="ExternalInput")
    wouthmb = nc.dram_tensor("wouthmb", [DH, H, D], BF16, kind="ExternalInput")
    b_g1 = nc.dram_tensor("b_g1", [G], F32, kind="ExternalInput")
    b_g2h = nc.dram_tensor("b_g2h", [D], F32, kind="ExternalInput")  # b_g2/2
    b_out = nc.dram_tensor("b_out", [D], F32, kind="ExternalInput")
    temp = nc.dram_tensor("temperature", [1], F32, kind="ExternalInput")
    outT = nc.dram_tensor("outT", [D, R], F32, kind="ExternalOutput")

    with tile.TileContext(nc) as tc:
        with (
            tc.tile_pool(name="const", bufs=1) as const,
            tc.tile_pool(name="work", bufs=2) as work,
            tc.tile_pool(name="ps_a", bufs=2, space="PSUM") as ps_a,
            tc.tile_pool(name="ps_acc", bufs=1, space="PSUM") as ps_acc,
            tc.tile_pool(name="ps_g", bufs=1, space="PSUM") as ps_g,
            tc.tile_pool(name="dram", bufs=1, space="DRAM") as dram,
        ):
            # ------------- load operands (k path first) -------------
            w_k_bf = const.tile([P, KC_D, D], BF16, name="w_k_bf")
            nc.sync.dma_start(out=w_k_bf[:, 0, :], in_=wkb[0:P, :])
            xT_bf = const.tile([P, KC_D, R], BF16, name="xT_bf", tag="slotx")
            nc.sync.dma_start(out=xT_bf[:, 0, :], in_=xTb[0:P, :])
            nc.sync.dma_start(
                out=w_k_bf[:, 1:, :],
                in_=wkb[P:, :].rearrange("(c p) n -> p c n", p=P),
            )
            nc.sync.dma_start(
                out=xT_bf[:, 1:, :],
                in_=xTb[P:, :].rearrange("(c p) n -> p c n", p=P),
            )
            w_q_bf = const.tile([P, KC_D, D], BF16, name="w_q_bf")
            nc.sync.dma_start(
                out=w_q_bf[:], in_=wqb[:].rearrange("(c p) n -> p c n", p=P)
            )
            exp_b_sb = const.tile([P, 1], F32, name="exp_b_sb")
            nc.vector.memset(exp_b_sb[:], EXP_BIAS)

            # ------------- k local projection + AllGather (split halves) ----
            FK2 = FLATK // 2
            ag_in_k1 = dram.tile([FK2], BF16, name="ag_in_k1")
            ag_in_k2 = dram.tile([FK2], BF16, name="ag_in_k2")
            ag_out_k1 = dram.tile([FLATK], BF16, name="ag_out_k1")
            ag_out_k2 = dram.tile([FLATK], BF16, name="ag_out_k2")
            ag_in_v = dram.tile([FLATV], FP8, name="ag_in_v")
            ag_out_v = dram.tile([2 * FLATV], FP8, name="ag_out_v")
            ag_in_vr = ag_in_v[:].rearrange("(r f) -> r f", f=D)

            kTl_bf = const.tile([P, H, R], BF16, name="kTl_bf", tag="slot16a")

            def k_head(h):
                psk = ps_a.tile([P, R], F32, tag="ps", name=f"psk{h}")
                for qn in range(2):
                    for kc in range(KC_D):
                        nc.tensor.matmul(
                            psk[:DH, qn * 512:(qn + 1) * 512],
                            w_k_bf[:, kc, h * DH:(h + 1) * DH],
                            xT_bf[:, kc, qn * 512:(qn + 1) * 512],
                            start=(kc == 0), stop=(kc == KC_D - 1),
                        )
                nc.vector.tensor_copy(out=kTl_bf[:DH, h, :], in_=psk[:DH, :])

            def k_head_qn(h, qn):
                # intra-window variant: single qn-half on the ps_g pool
                psk = ps_g.tile([P, R], F32, tag="psg", name=f"pskq{h}_{qn}")
                for kc in range(KC_D):
                    nc.tensor.matmul(
                        psk[:DH, 0:512],
                        w_k_bf[:, kc, h * DH:(h + 1) * DH],
                        xT_bf[:, kc, qn * 512:(qn + 1) * 512],
                        start=(kc == 0), stop=(kc == KC_D - 1),
                    )
                nc.vector.tensor_copy(
                    out=kTl_bf[:DH, h, qn * 512:(qn + 1) * 512],
                    in_=psk[:DH, 0:512],
                )

            def k_send(ag_in, ag_out, h0):
                nc.sync.dma_start(
                    out=ag_in[:].rearrange("(h p k) -> p h k", p=DH, k=R),
                    in_=kTl_bf[:DH, h0:h0 + 4, :],
                )
                if SIM_NO_COLLECTIVE:
                    nc.sync.dma_start(out=ag_out[0:FK2], in_=ag_in[:])
                    nc.sync.dma_start(out=ag_out[FK2:FLATK], in_=ag_in[:])
                else:
                    nc.gpsimd.collective_compute(
                        "AllGather",
                        mybir.AluOpType.bypass,
                        replica_groups=REPLICA_GROUPS,
                        ins=[ag_in[:]],
                        outs=[ag_out[:]],
                    )

            for h in range(4):
                k_head(h)
            k_send(ag_in_k1, ag_out_k1, 0)

            # ------------- q projection head 0 (window opener) -------------
            qT_bf = const.tile([P, H, R], BF16, name="qT_bf", tag="slot16q")
            nc.gpsimd.memset(qT_bf[64:128, :, :], 0.0)

            def q_head(h):
                psq = ps_acc.tile([P, R], F32, tag="acc", name=f"psq{h}")
                for qn in range(2):
                    for kc in range(KC_D):
                        nc.tensor.matmul(
                            psq[:DH, qn * 512:(qn + 1) * 512],
                            w_q_bf[:, kc, h * DH:(h + 1) * DH],
                            xT_bf[:, kc, qn * 512:(qn + 1) * 512],
                            start=(kc == 0), stop=(kc == KC_D - 1),
                        )
                nc.vector.tensor_copy(out=qT_bf[:DH, h, :], in_=psq[:DH, :])

            def q_head_qn(h, qn):
                # intra-window variant: single qn-half on the ps_g pool
                psq = ps_g.tile([P, R], F32, tag="psg", name=f"psqq{h}_{qn}")
                for kc in range(KC_D):
                    nc.tensor.matmul(
                        psq[:DH, 0:512],
                        w_q_bf[:, kc, h * DH:(h + 1) * DH],
                        xT_bf[:, kc, qn * 512:(qn + 1) * 512],
                        start=(kc == 0), stop=(kc == KC_D - 1),
                    )
                nc.vector.tensor_copy(
                    out=qT_bf[:DH, h, qn * 512:(qn + 1) * 512],
                    in_=psq[:DH, 0:512],
                )

            q_head(0)

            # ---- full kT from AllGather: window-critical DMA chain ----
            kT_bf = const.tile([P, H, S], BF16, name="kT_bf")
            nc.gpsimd.memset(kT_bf[64:128, :, :], 0.0)

            def kt_fill(hg, ag_out):
                for s in range(2):
                    nc.sync.dma_start(
                        out=kT_bf[:DH, hg:hg + 4, s * R:(s + 1) * R],
                        in_=ag_out[s * FK2:(s + 1) * FK2].rearrange(
                            "(h p k) -> p h k", p=DH, k=R
                        ),
                    )

            kt_fill(0, ag_out_k1)
            # fp8 v-path operands (deferred so k-path DMAs go first)
            xT_8 = const.tile([P, 3, 2, R], FP8, name="xT_8")
            nc.sync.dma_start(out=xT_8[:], in_=xTb8[:])
            w_v_8 = const.tile([P, 3, 2, D], FP8, name="w_v_8")
            nc.sync.dma_start(out=w_v_8[:], in_=wvb8[:])

            # ------------- v local projection (fp8 DoubleRow) + AllGather ----
            for vb in range(16):  # DoubleRow needs dst base 0, M<=64
                psv = ps_a.tile([P, R], F32, tag="ps", name=f"psv{vb}")
                for ns, w in ((0, 512), (512, 128)):
                    for pc in range(3):
                        nc.tensor.matmul(
                            psv[:64, ns:ns + w],
                            xT_8[:, pc, :, vb * 64:(vb + 1) * 64],
                            w_v_8[:, pc, :, ns:ns + w],
                            start=(pc == 0), stop=(pc == 2),
                            perf_mode=DR,
                        )
                v_sb = work.tile([64, D], FP8, tag="probsT", bufs=4,
                                 name=f"v_sb{vb}")
                nc.vector.tensor_copy(out=v_sb[:], in_=psv[:64, :D])
                nc.sync.dma_start(
                    out=ag_in_vr[vb * 64:(vb + 1) * 64, :], in_=v_sb[:]
                )

            if SIM_NO_COLLECTIVE:
                nc.sync.dma_start(out=ag_out_v[0:FLATV], in_=ag_in_v[:])
                nc.sync.dma_start(out=ag_out_v[FLATV:2 * FLATV], in_=ag_in_v[:])
            else:
                nc.gpsimd.collective_compute(
                    "AllGather",
                    mybir.AluOpType.bypass,
                    replica_groups=REPLICA_GROUPS,
                    ins=[ag_in_v[:]],
                    outs=[ag_out_v[:]],
                )

            # last dim padded 81->82 so the kc-pair step (8*82=656) is %16==0
            # as DoubleRow's stationary AP requires.
            v_aug = const.tile([P, NK, H, 82], FP8, name="v_aug", tag="slot20")
            nc.gpsimd.memset(v_aug[:, :, :, DH:DH + 1], 1.0)
            for s in range(2):
                vsh = ag_out_v[s * FLATV:(s + 1) * FLATV].rearrange(
                    "(r f) -> r f", f=D
                )
                for rc in range(8):
                    nc.sync.dma_start(
                        out=v_aug[:, s * 8 + rc, :, 0:DH],
                        in_=vsh[rc * P:(rc + 1) * P, :].rearrange(
                            "p (h d) -> p h d", d=DH
                        ),
                    )


            # ------------- main self-attention -------------
            dram_hs_sums = dram.tile([H, R], BF16, name="dram_hs_sums")
            hs_keep = []

            def attn_head(h, fillers=()):
                fillers = list(fillers)
                pshs = ps_acc.tile([P, R], F32, tag="acc", name=f"pshs{h}")
                for c in range(NK // 2):
                    if fillers and c > 0:
                        budget = 1100
                        while fillers and budget > 0:
                            cost, fn = fillers.pop(0)
                            fn()
                            budget -= cost
                    pt = work.tile([P, 2, R], FP8, tag="probsT", bufs=4,
                                   name=f"pt{h}_{c}")
                    for j in range(2):
                        pslog = ps_a.tile([P, R], F32, tag="ps",
                                          name=f"pslog{h}_{c}_{j}")
                        kc = 2 * c + j
                        with tc.high_priority():
                            for qn in range(2):
                                nc.tensor.matmul(
                                    pslog[:, qn * 512:(qn + 1) * 512],
                                    kT_bf[:, h, kc * P:(kc + 1) * P],
                                    qT_bf[:, h, qn * 512:(qn + 1) * 512],
                                    start=True, stop=True,
                                )
                        nc.scalar.activation(out=pt[:, j, :], in_=pslog[:],
                                             func=AF.Exp, scale=SCALE,
                                             bias=exp_b_sb[:, 0:1])
                    with tc.high_priority():
                        for qn in range(2):
                            nc.tensor.matmul(
                                pshs[:DH + 1, qn * 512:(qn + 1) * 512],
                                v_aug[:, 2 * c:2 * c + 2, h, 0:DH + 1],
                                pt[:, :, qn * 512:(qn + 1) * 512],
                                start=(c == 0),
                                stop=(c == NK // 2 - 1),
                                perf_mode=DR,
                            )
                for cost, fn in fillers:
                    fn()
                hs_st = work.tile([P, R], BF16, tag="hs_keep", bufs=8,
                                  name=f"hs_st{h}")
                nc.vector.tensor_copy(out=hs_st[:DH + 1, :], in_=pshs[:DH + 1, :])
                if h < 6:
                    nc.sync.dma_start(out=dram_hs_sums[h], in_=hs_st[DH:DH + 1, :])
                hs_keep.append(hs_st)

            dram_rec_row = dram.tile([H, R], BF16, name="dram_rec_row")

            def row_recip_mul(sums_row, dst, src, h, name):
                """dst = src * (1/sums_row); recip on the [1,R] sums row, then
                a 2KB DRAM bounce for the partition broadcast. The sums row
                lives on partition 80, so it reaches partition 0 by DMA
                (engines cannot shift partitions)."""
                r16 = work.tile([1, R], BF16, tag="rr16", bufs=1,
                                name=f"{name}_16")
                nc.sync.dma_start(out=r16[:], in_=sums_row)
                rb = work.tile([1, R], BF16, tag="rrb", bufs=1, name=f"{name}_b")
                for qn in range(2):
                    sl = np.s_[:, qn * 512:(qn + 1) * 512]
                    rf = work.tile([1, 512], F32, tag="rrf", bufs=1,
                                   name=f"{name}_f{qn}")
                    nc.vector.tensor_copy(out=rf[:], in_=r16[sl])
                    ro = work.tile([1, 512], F32, tag="rro", bufs=1,
                                   name=f"{name}_o{qn}")
                    nc.vector.reciprocal_approx_fast(ro[:], rf[:])
                    nc.vector.tensor_copy(out=rb[sl], in_=ro[:])
                nc.sync.dma_start(out=dram_rec_row[h, :], in_=rb[0:1, :])
                bc = work.tile([DH, R], BF16, tag="bc", bufs=1, name=f"{name}_bc")
                nc.sync.dma_start(
                    out=bc[:], in_=dram_rec_row[h:h + 1, :].to_broadcast((DH, R))
                )
                nc.vector.tensor_mul(dst, src, bc[:])

            dram_au = dram.tile([H, DH + 1, R], BF16, name="dram_au")

            def au_head(h):
                psal = ps_g.tile([P, R], F32, tag="psg", name=f"psal{h}")
                for qn in range(2):
                    nc.tensor.matmul(
                        psal[:A, qn * 512:(qn + 1) * 512],
                        au_kT_s[:, h, :],
                        qT_bf[:, h, qn * 512:(qn + 1) * 512],
                        start=True, stop=True,
                    )
                nc.scalar.activation(out=au_pT[:A, :], in_=psal[:A, :],
                                     func=AF.Exp)
                psau = ps_g.tile([P, R], F32, tag="psg", name=f"psau{h}")
                for qn in range(2):
                    nc.tensor.matmul(
                        psau[:DH + 1, qn * 512:(qn + 1) * 512],
                        au_v_aug[:, h, :],
                        au_pT[:, qn * 512:(qn + 1) * 512],
                        start=True, stop=True,
                    )
                au_st = work.tile([P, R], BF16, tag="evac", bufs=1,
                                  name=f"au_st{h}")
                nc.vector.tensor_copy(out=au_st[:DH + 1, :], in_=psau[:DH + 1, :])
                nc.sync.dma_start(out=dram_au[h], in_=au_st[:DH + 1, :])

            # reciprocal chain: per-(head,query) sums -> 1/sum (bf16) in DRAM
            def recip_chain(sums_src, name, dma_eng):
                rc_in = work.tile([P, 64], BF16, tag="rc", bufs=1, name=f"{name}_in")
                for h in range(H):
                    dma_eng.dma_start(
                        out=rc_in[h * 16:(h + 1) * 16, :],
                        in_=sums_src(h),
                    )
                rc_f = work.tile([P, 64], F32, tag="rcf", bufs=1, name=f"{name}_f")
                nc.vector.tensor_copy(out=rc_f[:], in_=rc_in[:])
                rc_s = work.tile([P, 64], F32, tag="rcs", bufs=1, name=f"{name}_s")
                rc_o = work.tile([P, 64], F32, tag="rco", bufs=1, name=f"{name}_o")
                nc.vector.reciprocal_approx_accurate(rc_o[:], rc_f[:], rc_s[:])
                rc_b = work.tile([P, 64], BF16, tag="rcb", bufs=1, name=f"{name}_b")
                nc.vector.tensor_copy(out=rc_b[:], in_=rc_o[:])
                drec = dram.tile([H, R], BF16, name=f"{name}_dr")
                dma_eng.dma_start(
                    out=drec[:].rearrange("h (a j) -> (h a) j", j=64), in_=rc_b[:]
                )
                return drec

            au_rec_ref = []

            def au_finish():
                dram_au_rec = recip_chain(
                    lambda h: dram_au[h, DH, :].rearrange("(a j) -> a j", j=64),
                    "aurec", nc.sync,
                )
                nc.gpsimd.memset(au_hsT[64:128, :, :], 0.0)
                for h in range(H):
                    bc = work.tile([DH, R], BF16, tag="bc", bufs=1, name=f"aubc{h}")
                    nc.sync.dma_start(
                        out=bc[:], in_=dram_au_rec[h:h + 1, :].to_broadcast((DH, R))
                    )
                    au_ld = work.tile([DH, R], BF16, tag="evac", bufs=1,
                                      name=f"auld{h}")
                    nc.sync.dma_start(out=au_ld[:], in_=dram_au[h, 0:DH, :])
                    nc.vector.tensor_mul(au_hsT[:DH, h, :], au_ld[:], bc[:])
                au_rec_ref.append(dram_au_rec)

            au_hsT = const.tile([P, H, R], BF16, name="au_hsT", tag="slot16a")

            # ---- attention ladder: projections/AU heads pumped as <=1.1us
            # filler quanta inside each head's chunk loop so the exp stream
            # never drains the pslog double-buffer. ----
            attn_head(0, [
                (1070, lambda: k_head_qn(4, 0)), (1070, lambda: k_head_qn(4, 1)),
                (1070, lambda: k_head_qn(5, 0)), (1070, lambda: k_head_qn(5, 1)),
                (1070, lambda: k_head_qn(6, 0)), (1070, lambda: k_head_qn(6, 1)),
                (1070, lambda: k_head_qn(7, 0)), (1070, lambda: k_head_qn(7, 1)),
                (600, lambda: (k_send(ag_in_k2, ag_out_k2, 4),
                               kt_fill(4, ag_out_k2))),
                (1070, lambda: q_head_qn(1, 0)), (1070, lambda: q_head_qn(1, 1)),
            ])
            # AU operands (deferred loads)
            w_ak_bf = const.tile([P, KC_C, D], BF16, name="w_ak_bf")
            nc.sync.dma_start(
                out=w_ak_bf[:], in_=wakb[:].rearrange("(c p) n -> p c n", p=P)
            )
            w_av_bf = const.tile([P, KC_C, D], BF16, name="w_av_bf")
            nc.sync.dma_start(
                out=w_av_bf[:], in_=wavb[:].rearrange("(c p) n -> p c n", p=P)
            )
            auT_bf = const.tile([P, KC_C, A], BF16, name="auT_bf")
            nc.sync.dma_start(
                out=auT_bf[:], in_=auTb[:].rearrange("(c p) n -> p c n", p=P)
            )
            t_sb = const.tile([P, 1], F32, name="t_sb")
            nc.sync.dma_start(out=t_sb[:], in_=temp[:].to_broadcast((P, 1)))
            alpha_s = const.tile([P, 1], F32, name="alpha_s")
            nc.vector.tensor_scalar_mul(alpha_s[:], t_sb[:], SCALE)

            # ------------- AU cross-attention constants -------------
            au_kT_s = const.tile([P, H, A], BF16, name="au_kT_s")
            nc.gpsimd.memset(au_kT_s[64:128, :, :], 0.0)
            for h in range(H):
                psak = ps_g.tile([P, R], F32, tag="psg", name=f"psak{h}")
                for kc in range(KC_C):
                    nc.tensor.matmul(
                        psak[:DH, 0:A],
                        w_ak_bf[:, kc, h * DH:(h + 1) * DH],
                        auT_bf[:, kc, :],
                        start=(kc == 0), stop=(kc == KC_C - 1),
                    )
                nc.vector.tensor_scalar_mul(
                    au_kT_s[:DH, h, :], psak[:DH, 0:A], alpha_s[:DH]
                )

            au_v_aug = const.tile([P, H, DH + 1], BF16, name="au_v_aug")
            nc.gpsimd.memset(au_v_aug[:], 0.0)
            nc.gpsimd.memset(au_v_aug[:A, :, DH:DH + 1], 1.0)
            psav = ps_g.tile([P, R], F32, tag="psg", name="psav")
            for ns, w in ((0, 512), (512, 128)):
                for kc in range(KC_C):
                    nc.tensor.matmul(
                        psav[:A, ns:ns + w],
                        auT_bf[:, kc, :],
                        w_av_bf[:, kc, ns:ns + w],
                        start=(kc == 0), stop=(kc == KC_C - 1),
                    )
            nc.vector.tensor_copy(
                out=au_v_aug[:A, :, 0:DH],
                in_=psav[:A, 0:D].rearrange("p (h d) -> p h d", d=DH),
            )

            # persistent AU probs tile: zeroed once, exps rewrite rows 0:16
            au_pT = const.tile([P, R], BF16, name="au_pT")
            nc.gpsimd.memset(au_pT[:, :], 0.0)
            attn_head(1, [
                (900, lambda: au_head(0)), (900, lambda: au_head(1)),
                (1070, lambda: q_head_qn(2, 0)), (1070, lambda: q_head_qn(2, 1)),
            ])
            attn_head(2, [
                (1070, lambda: q_head_qn(3, 0)), (1070, lambda: q_head_qn(3, 1)),
                (900, lambda: au_head(2)), (900, lambda: au_head(3)),
                (1070, lambda: q_head_qn(4, 0)), (1070, lambda: q_head_qn(4, 1)),
            ])
            attn_head(3, [
                (1070, lambda: q_head_qn(5, 0)), (1070, lambda: q_head_qn(5, 1)),
                (900, lambda: au_head(4)), (900, lambda: au_head(5)),
                (1070, lambda: q_head_qn(6, 0)), (1070, lambda: q_head_qn(6, 1)),
            ])
            attn_head(4, [
                (1070, lambda: q_head_qn(7, 0)), (1070, lambda: q_head_qn(7, 1)),
                (900, lambda: au_head(6)), (900, lambda: au_head(7)),
            ])
            au_finish()

            # late loads: gate + out-proj weights (DMA slack mid-window)
            w_g1_hm = const.tile([P, H, G], BF16, name="w_g1_hm")
            nc.gpsimd.memset(w_g1_hm[64:128, :, :], 0.0)
            nc.sync.dma_start(out=w_g1_hm[:DH, :, :], in_=wg1hmb[:])
            w_g2_bf = const.tile([P, 3, D], BF16, name="w_g2_bf")
            nc.sync.dma_start(
                out=w_g2_bf[:, 0:2, :],
                in_=wg2b[0:256, :].rearrange("(c p) n -> p c n", p=P),
            )
            nc.sync.dma_start(out=w_g2_bf[:64, 2, :], in_=wg2b[256:320, :])
            nc.gpsimd.memset(w_g2_bf[64:128, 2, :], 0.0)
            w_out_hm = const.tile([P, H, D], BF16, name="w_out_hm")
            nc.gpsimd.memset(w_out_hm[64:128, :, :], 0.0)
            nc.sync.dma_start(out=w_out_hm[:DH, :, :], in_=wouthmb[:])
            b_g1_sb = const.tile([P, 3], F32, name="b_g1_sb")
            nc.vector.memset(b_g1_sb[:], 0.0)
            nc.sync.dma_start(
                out=b_g1_sb[:, 0:2], in_=b_g1[0:256].rearrange("(c p) -> p c", p=P)
            )
            nc.sync.dma_start(out=b_g1_sb[:64, 2:3], in_=b_g1[256:320][:, None])
            b_g2_hm = const.tile([P, H], F32, name="b_g2_hm")  # holds b_g2/2
            nc.vector.memset(b_g2_hm[:], 0.0)
            nc.sync.dma_start(
                out=b_g2_hm[:DH, :], in_=b_g2h[:].rearrange("(h p) -> p h", p=DH)
            )
            b_out_sb = const.tile([P, KC_D], F32, name="b_out_sb")
            nc.sync.dma_start(
                out=b_out_sb[:], in_=b_out[:].rearrange("(c p) -> p c", p=P)
            )

            # residual+bias rows preloaded so out-proj finishes don't wait DMA
            rx_t = {}
            for mo in range(KC_D):
                rx = work.tile([P, R], F32, tag="rx", bufs=3, name=f"rx{mo}")
                nc.sync.dma_start(out=rx[:], in_=xT[mo * P:(mo + 1) * P, :])
                nc.vector.tensor_scalar_add(rx[:], rx[:], b_out_sb[:, mo:mo + 1])
                rx_t[mo] = rx

            # ---- gate MLP (emitted here; runs under attn3-5 exp windows) ----
            siluT = const.tile([P, 3, R], BF16, name="siluT", tag="slot16q_silu")
            nc.gpsimd.memset(siluT[64:128, 2, :], 0.0)

            def l1_q(mo, qn):
                rows = 128 if mo < 2 else 64
                psl1 = ps_g.tile([P, R], F32, tag="psg", name=f"psl1{mo}_{qn}")
                for h in range(H):
                    nc.tensor.matmul(
                        psl1[:rows, 0:512],
                        w_g1_hm[:, h, mo * P:mo * P + rows],
                        au_hsT[:, h, qn * 512:(qn + 1) * 512],
                        start=(h == 0), stop=(h == H - 1),
                    )
                nc.scalar.activation(
                    out=siluT[:rows, mo, qn * 512:(qn + 1) * 512],
                    in_=psl1[:rows, 0:512],
                    func=AF.Silu, scale=0.125, bias=b_g1_sb[:rows, mo:mo + 1],
                )

            fusedA = const.tile([P, 6, R], BF16, name="fusedA", tag="slotx")
            nc.gpsimd.memset(fusedA[64:128, :, :], 0.0)
            fusedB6 = const.tile([P, R], BF16, name="fusedB6")
            nc.gpsimd.memset(fusedB6[64:128, :], 0.0)
            fusedB7 = const.tile([P, R], BF16, name="fusedB7")
            nc.gpsimd.memset(fusedB7[64:128, :], 0.0)

            def fused_sl(h):
                if h < 6:
                    return fusedA[:, h, :]
                return fusedB6[:, :] if h == 6 else fusedB7[:, :]

            def gate_head(h):
                # tanh(psg/2 + b_g2/2); gate*au_hs = (1+t)*au_hs/2
                psg = ps_g.tile([P, R], F32, tag="psg", name=f"psgate{h}")
                for qn in range(2):
                    for kc in range(3):
                        nc.tensor.matmul(
                            psg[:DH, qn * 512:(qn + 1) * 512],
                            w_g2_bf[:, kc, h * DH:(h + 1) * DH],
                            siluT[:, kc, qn * 512:(qn + 1) * 512],
                            start=(kc == 0), stop=(kc == 2),
                        )
                gateT = work.tile([DH, R], BF16, tag="gateT", bufs=1,
                                  name=f"gateT{h}")
                nc.scalar.activation(
                    out=gateT[:], in_=psg[:DH, :],
                    func=AF.Tanh, scale=0.5, bias=b_g2_hm[:DH, h:h + 1],
                )
                # fused = (gateT + 1) * au_hsT  (au_hsT carries x8 => x16 net)
                nc.vector.scalar_tensor_tensor(
                    out=fused_sl(h)[:DH, :], in0=gateT[:], scalar=1.0,
                    in1=au_hsT[:DH, h, :], op0=ALU.add, op1=ALU.mult,
                )

            attn_head(5, [
                (1700, lambda mo=mo, qn=qn: l1_q(mo, qn))
                for mo in range(3) for qn in range(2)
            ])
            attn_head(6, [
                (1300, lambda h=h: gate_head(h)) for h in range(H)
            ])

            # ---- hs recip chain A: heads 0..5; fused(h) += hs*rec ----
            rcA_in = work.tile([P, 64], BF16, tag="rc", bufs=1, name="rcA_in")
            for h in range(6):
                nc.sync.dma_start(
                    out=rcA_in[h * 16:(h + 1) * 16, :],
                    in_=dram_hs_sums[h, :].rearrange("(a j) -> a j", j=64),
                )
            rcA_f = work.tile([P, 64], F32, tag="rcf", bufs=1, name="rcA_f")
            nc.vector.memset(rcA_f[96:, :], 1.0)
            nc.vector.tensor_copy(out=rcA_f[:96, :], in_=rcA_in[:96, :])
            rcA_s = work.tile([P, 64], F32, tag="rcs", bufs=1, name="rcA_s")
            rcA_o = work.tile([P, 64], F32, tag="rco", bufs=1, name="rcA_o")
            nc.vector.reciprocal_approx_accurate(rcA_o[:], rcA_f[:], rcA_s[:])
            rcA_b = work.tile([P, 64], BF16, tag="rcb", bufs=1, name="rcA_b")
            nc.vector.tensor_copy(out=rcA_b[:96, :], in_=rcA_o[:96, :])
            dram_hs_rec = dram.tile([H, R], BF16, name="hsrec_dr")
            nc.sync.dma_start(
                out=dram_hs_rec[0:6, :].rearrange("h (a j) -> (h a) j", j=64),
                in_=rcA_b[:96, :],
            )
            for h in range(6):
                bch = work.tile([DH, R], BF16, tag="bc", bufs=1, name=f"hsbc{h}")
                nc.sync.dma_start(
                    out=bch[:], in_=dram_hs_rec[h:h + 1, :].to_broadcast((DH, R))
                )
                hs_st = hs_keep[h]
                nc.vector.tensor_mul(hs_st[:DH, :], hs_st[:DH, :], bch[:])
                nc.vector.tensor_add(
                    fused_sl(h)[:DH, :], fused_sl(h)[:DH, :], hs_st[:DH, :]
                )

            # ---- per-head tail recip for h6/h7 ----
            def recip_tail(h):
                # fast SBUF-only recip: sums row is hs_st row 80
                row_recip_mul(
                    hs_keep[h][DH:DH + 1, :], hs_keep[h][:DH, :],
                    hs_keep[h][:DH, :], h, f"rt{h}",
                )
                nc.vector.tensor_add(
                    fused_sl(h)[:DH, :], fused_sl(h)[:DH, :], hs_keep[h][:DH, :]
                )

            recip_tail(6)

            # ------------- output projection + residual -------------
            pso_t = {}

            PSO_POOL = {0: (ps_g, "psg"), 1: (ps_a, "ps"), 2: (ps_acc, "acc"),
                        3: (ps_a, "ps"), 4: (ps_g, "psg")}

            def out_partial(mo, h0, h1):
                if mo not in pso_t:
                    pool, tg = PSO_POOL[mo]
                    pso_t[mo] = pool.tile([P, R], F32, tag=tg, name=f"pso{mo}")
                t = pso_t[mo]
                for qn in range(2):
                    for h in range(h0, h1):
                        nc.tensor.matmul(
                            t[:, qn * 512:(qn + 1) * 512],
                            w_out_hm[:, h, mo * P:(mo + 1) * P],
                            fused_sl(h)[:, qn * 512:(qn + 1) * 512],
                            start=(h == 0), stop=(h == H - 1),
                        )

            def out_finish(mo):
                # out = pso/16 + (residual + b_out), in place on the rx tile
                rx = rx_t[mo]
                nc.vector.scalar_tensor_tensor(
                    out=rx[:], in0=pso_t[mo][:], scalar=0.0625,
                    in1=rx[:], op0=ALU.mult, op1=ALU.add,
                )
                nc.sync.dma_start(out=outT[mo * P:(mo + 1) * P, :], in_=rx[:])

            attn_head(7)
            # h0-6 partials for mo 0,1 run under attn7's exp window
            out_partial(0, 0, 7)
            out_partial(1, 0, 7)

            recip_tail(7)

            out_partial(0, 7, H)
            out_finish(0)
            for mo in range(2, KC_D):
                out_partial(mo, 0, 7)
                out_partial(mo - 1, 7, H)
                out_finish(mo - 1)
            out_partial(KC_D - 1, 7, H)
            out_finish(KC_D - 1)

    nc.finalize()
    return nc


_NC_CACHE = []


def get_program():
    if not _NC_CACHE:
        _NC_CACHE.append(_build_program())
    return _NC_CACHE[0]


def _bf(x):
    import ml_dtypes
    return np.ascontiguousarray(x.astype(ml_dtypes.bfloat16))


def _f8(x):
    import ml_dtypes
    return np.ascontiguousarray(x.astype(ml_dtypes.float8_e4m3))


def _pair_chunks(w, rows, cols, scale):
    """[rows<=768, cols] -> fp8 [128, 3, 2, cols] over d-chunk pairs."""
    out = np.zeros((P, 3, 2, cols), np.float32)
    for pc in range(3):
        for i in range(2):
            r0 = (2 * pc + i) * P
            r1 = min(r0 + P, rows)
            if r0 < rows:
                out[: r1 - r0, pc, i, :] = w[r0:r1, :]
    return _f8(out * scale)


def kernel(**inputs):
    f = lambda k: np.ascontiguousarray(np.asarray(inputs[k], dtype=np.float32))
    hidden = f("hidden_states")          # [4, 2048, 640]
    au = f("au_embedding")               # [4, 16, 768]
    w_g1 = f("w_g1")                     # [640, 320]
    w_out_w = f("w_out")                 # [640, 640]
    shared = {
        "wqb": _bf(f("w_q")),
        "wkb": _bf(f("w_k")),
        "wvb8": _pair_chunks(f("w_v"), D, D, 16.0),
        "wakb": _bf(f("w_ak")),
        "wavb": _bf(f("w_av") * 8.0),
        "wg1hmb": _bf(w_g1.reshape(H, DH, G).transpose(1, 0, 2)),
        "wg2b": _bf(f("w_g2")),
        "wouthmb": _bf(w_out_w.reshape(H, DH, D).transpose(1, 0, 2)),
        "b_g1": f("b_g1"),
        "b_g2h": f("b_g2") * 0.5,
        "b_out": f("b_out"),
        "temperature": f("temperature"),
    }
    in_maps = []
    for c in range(N_CORES):
        b, half = divmod(c, 2)
        m = dict(shared)
        xt = np.ascontiguousarray(hidden[b, half * R:(half + 1) * R, :].T)
        m["xT"] = xt
        m["xTb"] = _bf(xt)
        m["xTb8"] = _pair_chunks(xt, D, R, 1.0)
        m["auTb"] = _bf(np.ascontiguousarray(au[b].T))
        in_maps.append(m)

    nc = get_program()
    try:
        res = run_bass_kernel_spmd(nc, in_maps, core_ids=list(range(N_CORES)))
    except Exception:
        # transient device wedge (NRT_EXEC_UNIT_UNRECOVERABLE) — retry once
        import time as _time
        _time.sleep(10)
        res = run_bass_kernel_spmd(nc, in_maps, core_ids=list(range(N_CORES)))

    out = np.empty((B, S, D), dtype=np.float32)
    for c in range(N_CORES):
        b, half = divmod(c, 2)
        out[b, half * R:(half + 1) * R, :] = res.results[c]["outT"].T
    return out
